# revision 1
# baseline (speedup 1.0000x reference)
"""Causal self-attention kernel for 8 Trainium2 NeuronCores.

Problem (hardcoded): x [4, 2048, 1024], torch-style Linear weights
W_q/W_k/W_v/W_o [1024, 1024], b_o [1024]; 16 heads, head_dim 64,
causal softmax attention, out = attn(x) @ W_o.T + b_o.

Sharding: 8 cores = 4 batches x 2 head-groups (8 heads each).
Each core computes a partial output  y_g @ W_o[:, g].T  for its batch;
the host sums the two head-group partials, rescales, and adds b_o.

Per-core pipeline:
  phase A: QKV projections in fp8e4m3 DoubleRow (2 contraction rows per
           partition, 0.5 PE cycles/row) with two-sided error
           compensation: x ~ x_hi + x_lo, W ~ W_hi + W_lo (all fp8,
           host-split, pre-scaled by 8/64 into e4m3 range), accumulate
           x_hi*W_hi + x_lo*W_hi + x_hi*W_lo in PSUM (12 matmuls per
           512-wide tile = 0.75x the fp16 cost).  q/k land as qT/kT
           [dq, T] fp16; v natural [tok, h, 65] fp16 with ones column,
           produced per head-pair so PV(0)/PV(1) unblock early (they
           gate the exp pipeline via ep-buffer recycling).
  phase B: per head: scores S^T[k, q] fp16 (contraction dh=64) ->
           exp on ACT (scale 2^-21 folds softmax scale and the fp8
           pre-scales) into fp16 ep tiles [keys, q-span], causal span
           exact at 128 granularity; diagonal blocks masked via
           triangular mask-multiply on Pool (SBUF only - Pool cannot
           touch PSUM on TRN2).
  phase C: PV with moving dim = dh: y[q, 65] = sum_r ep_r^T @ v_r
           (65 PE cycles per causal block); col 64 is the softmax
           denominator; normalize via DVE reciprocal + DVE
           tensor_scalar (per-partition scalar, extra 0.25 fold);
           head-pair transposes on PE into yT (PSUM->SBUF move by DMA).
  phase D: out accumulated in fp16 SBUF: stage A fuses yT_{0,1,2} @ Wo
           in one PSUM group after pair 2, stage B adds yT_3 @ Wo in a
           per-query-tile tail pipeline woven with PV(7)/transpose,
           DMA out fp16 on both queues (host rescales by 1/128).

Emission order is schedule-driven (SCHEDULE below) so per-engine
program order can be tuned against the cost model.
"""

import numpy as np
import ml_dtypes

import concourse.bass as bass
import concourse.tile as tile
import concourse.mybir as mybir
from concourse import bacc
from concourse import bass_utils

T = 2048
D = 1024
HPC = 8            # heads per core
DH = 64
DQ = HPC * DH      # 512, per-core projection width
NT = T // 128      # 16 row tiles
NJ = DQ // 128     # 4 dq tiles

F32 = mybir.dt.float32
F16 = mybir.dt.float16
F8 = mybir.dt.float8e4
EXP = mybir.ActivationFunctionType.Exp
DR = mybir.MatmulPerfMode.DoubleRow
MUL = mybir.AluOpType.mult
E4NP = ml_dtypes.float8_e4m3

X_SCALE = 8.0      # host pre-scale of x before fp8 split
W_SCALE = 64.0     # host pre-scale of W_{q,k,v} before fp8 split
EXP_SCALE = 0.125 / (X_SCALE * W_SCALE) ** 2   # == 2^-21, exact
Y_FOLD = 0.25                                  # extra fold in normalize
OUT_DESCALE = 1.0 / (X_SCALE * W_SCALE * Y_FOLD)   # host undo (1/128)

TRACE = False
LAST = None        # BassKernelResults of the most recent run

TRIMASK = np.triu(np.ones((128, 128), dtype=np.float16))
IDENT = np.eye(128, dtype=np.float16)

# ep tag bufs: big tiles (small r) need depth 3 so exp(h) is only gated
# by PV(h-3); late-emitted small tiles can ride at depth 2 (their gate,
# PV(h-2)-end, passes long before their emission slot).
EP_BUFS = {r: (3 if r < 8 else 2) for r in range(NT)}

# Emission order (per-engine program order).  Ops:
#   ("qk", j)         Q+K projection, dq-tile j (c-descending)
#   ("s", h)          scores + exp + diag mask, head h
#   ("vp", p)         V projection, head pair p (all 16 token tiles)
#   ("pv", h)         attention*V + normalize, head h
#   ("tp", j)         pair transpose into yT_j (j <= 2)
#   ("oja",)          out-proj stage A: yT_{0,1,2} fused PSUM groups
#   ("tail",)         woven PV(7)+transpose+out-proj stage B+DMA
#   ("close_wqk",) ("close_x8",) ("open_late",)   pool scope markers
SCHEDULE = [
    ("qk", 0), ("s", 0, 0, NT),
    ("qk", 1), ("s", 1, 0, NT),
    ("vp", 0), ("pv", 0),
    ("s", 2, 0, 2),
    ("qk", 2), ("s", 2, 2, NT),
    ("s", 3, 0, 2),
    ("qk", 3), ("s", 3, 2, NT),
    ("close_wqk",),
    ("vp", 1), ("pv", 1),
    ("s", 4, 0, NT),
    ("vp", 2), ("pv", 2),
    ("s", 5, 0, NT),
    ("vp", 3), ("pv", 3),
    ("close_x8",), ("open_late",),
    ("tp", 0), ("tp", 1),
    ("s", 6, 0, NT), ("pv", 4),
    ("s", 7, 0, NT), ("pv", 5),
    ("tp", 2),
    ("pv", 6),
    ("tail",),
]


def _split8(a):
    """Two-sided fp8 split: a ~ hi + lo, both e4m3."""
    hi = np.clip(a, -240, 240).astype(E4NP)
    lo = np.clip(a - hi.astype(np.float32), -240, 240).astype(E4NP)
    return hi, lo


def _body(tc):
    nc = tc.nc
    # fp8 x: [c, p, s, i, col]: row 256s+128i+p of x.T*8, col 512c+col
    x8h_d = nc.dram_tensor("x8h", (4, 128, 4, 2, 512), F8, kind="ExternalInput").ap()
    x8l_d = nc.dram_tensor("x8l", (4, 128, 4, 2, 512), F8, kind="ExternalInput").ap()
    w8_d = {}
    for nm in ("wqh", "wql", "wkh", "wkl", "wvh", "wvl"):
        w8_d[nm] = nc.dram_tensor(nm, (128, 4, 2, DQ), F8, kind="ExternalInput").ap()
    wo_d = nc.dram_tensor("wot", (DQ, D), F16, kind="ExternalInput").ap()
    tm_d = nc.dram_tensor("trimask", (128, 128), F16, kind="ExternalInput").ap()
    id_d = nc.dram_tensor("ident", (128, 128), F16, kind="ExternalInput").ap()
    out_d = nc.dram_tensor("out", (T, D), F16, kind="ExternalOutput").ap()

    st = {}  # emission state

    def emit_qk(j):
        x8h, x8l, w8 = st["x8h"], st["x8l"], st["w8"]
        for (wh, wl, dest) in ((w8["wqh"], w8["wql"], st["qT"]),
                               (w8["wkh"], w8["wkl"], st["kT"])):
            terms = ((x8h, wh), (x8l, wh), (x8h, wl))
            for c in range(3, -1, -1):  # c-descending
                ps = st["flex"].tile([128, 512], F32, tag="fx", name=f"p{j}_{c}")
                ki = 0
                for (xs, ws) in terms:
                    for s in range(4):
                        nc.tensor.matmul(
                            ps[:],
                            ws[:, s, :, 128 * j:128 * (j + 1)],
                            xs[:, s, :, 512 * c:512 * (c + 1)],
                            start=(ki == 0), stop=(ki == 11),
                            perf_mode=DR,
                        )
                        ki += 1
                nc.vector.tensor_copy(dest[:, j, 512 * c:512 * (c + 1)], ps[:])

    def emit_vp(p):
        # V projection for head pair p: out columns [128p, 128p+128)
        x8h, x8l, w8v = st["x8h"], st["x8l"], st["w8v"]
        terms = ((x8h, w8v["wvh"]), (x8l, w8v["wvh"]), (x8h, w8v["wvl"]))
        for t in range(NT):
            ps = st["flex"].tile([128, 128], F32, tag="fx", name=f"pv{p}_{t}")
            ki = 0
            for (xs, ws) in terms:
                for s in range(4):
                    nc.tensor.matmul(
                        ps[:],
                        xs[:, s, :, 128 * t:128 * (t + 1)],
                        ws[:, s, :, 128 * p:128 * (p + 1)],
                        start=(ki == 0), stop=(ki == 11),
                        perf_mode=DR,
                    )
                    ki += 1
            nc.vector.tensor_copy(
                st["v"][:, t, 2 * p:2 * p + 2, 0:DH],
                ps[:].rearrange("p (h d) -> p h d", h=2),
            )

    def emit_s(h, r0=0, r1=NT):
        qT, kT, trim = st["qT"], st["kT"], st["trim"]
        hp = (h % 2) * DH
        hj = h // 2
        for r in range(r0, r1):
            s0 = 128 * r
            ep = st["epool"].tile([128, T - s0], F16, tag=f"e{r}",
                                  name=f"e{r}_h{h}", bufs=EP_BUFS[r])
            st["ep"][(h, r)] = ep
            bounds = ([(s0, 1024), (1024, 2048)] if s0 < 1024
                      else [(s0, 2048)])
            for (lo, hi) in bounds:
                stt = st["stp"].tile([128, hi - lo], F32, tag="st")
                for n0 in range(lo, hi, 512):
                    n1 = min(n0 + 512, hi)
                    nc.tensor.matmul(
                        stt[:, n0 - lo:n1 - lo],
                        kT[hp:hp + DH, hj, s0:s0 + 128],
                        qT[hp:hp + DH, hj, n0:n1],
                        start=True, stop=True,
                    )
                nc.scalar.activation(
                    ep[:, lo - s0:hi - s0], stt[:], EXP, scale=EXP_SCALE,
                )
            nc.gpsimd.tensor_mul(ep[:, 0:128], ep[:, 0:128], trim[:])

    def pv_tile(h, qt):
        """One PV psum group + normalize for (head, query tile)."""
        v, yn = st["v"], st["yn"]
        par = (h // 2) % 2
        hq = h % 2
        ya = st["flex"].tile([128, DH + 1], F32, tag="fx", name=f"y{h}_{qt}")
        for r in range(qt + 1):
            ep = st["ep"][(h, r)]
            off = 128 * (qt - r)
            nc.tensor.matmul(
                ya[:], ep[:, off:off + 128], v[:, r, h, :],
                start=(r == 0), stop=(r == qt),
            )
        rec = st["sp"].tile([128, 1], F32, tag="rec", name=f"rec{h}_{qt}")
        nc.vector.reciprocal(rec[:], ya[:, DH:DH + 1])
        nc.vector.tensor_scalar(
            yn[:, par, qt, hq, :], ya[:, 0:DH], rec[:], Y_FOLD, MUL, MUL,
        )

    def emit_pv(h):
        for qt in range(NT):
            pv_tile(h, qt)
        for r in range(NT):
            del st["ep"][(h, r)]

    def tp_tile(j, qt, yTj):
        tp = st["flex"].tile([128, 128], F16, tag="fx", name=f"tp{j}_{qt}")
        nc.tensor.transpose(tp[:], st["yn"][:, j % 2, qt, :, :], st["ident"][:])
        nc.vector.tensor_copy(yTj[:, 128 * qt:128 * (qt + 1)], tp[:])

    def emit_tp(j):
        yTj = st["ytp"].tile([128, T], F16, tag="yT", name=f"yT{j}")
        st["yT"][j] = yTj
        for qt in range(NT):
            tp_tile(j, qt, yTj)

    def emit_tail():
        # Software-pipelined tail, lag 2: stage 1 at qt = PV(7, qt) +
        # pair-3 transpose into yT3[qt]; stage 2 at qt-2 = the WHOLE
        # out projection (one 4-matmul psum group per (qt, half)) ->
        # single psum->sbuf copy -> DMA out.  The lag decouples PE from
        # the DVE copy chain (PSUM slots recycle two tiles back).
        woT = st["woT"]
        yT3s = {}

        def oj_qt(q):
            for d in range(2):
                ps = st["flex"].tile([128, 512], F32, tag="fx",
                                     name=f"ob{q}_{d}")
                for j in range(4):
                    lhs = (st["yT"][j][:, 128 * q:128 * (q + 1)]
                           if j < 3 else yT3s[q][:])
                    nc.tensor.matmul(
                        ps[:], lhs, woT[:, j, 512 * d:512 * (d + 1)],
                        start=(j == 0), stop=(j == 3),
                    )
                ob = st["lp"].tile([128, 512], F16, tag="ob",
                                   name=f"ob{q}_{d}", bufs=4)
                nc.vector.tensor_copy(ob[:], ps[:])
                qu = nc.sync if d == 0 else nc.gpsimd
                qu.dma_start(
                    out_d[128 * q:128 * (q + 1), 512 * d:512 * (d + 1)],
                    ob[:])
            del yT3s[q]

        for qt in range(NT):
            pv_tile(7, qt)
            yT3s[qt] = st["lp"].tile([128, 128], F16, tag="yt3",
                                     name=f"yt3_{qt}", bufs=4)
            tp = st["flex"].tile([128, 128], F16, tag="fx", name=f"tp3_{qt}")
            nc.tensor.transpose(tp[:], st["yn"][:, 1, qt, :, :], st["ident"][:])
            nc.vector.tensor_copy(yT3s[qt][:], tp[:])
            if qt >= 2:
                oj_qt(qt - 2)
        oj_qt(NT - 2)
        oj_qt(NT - 1)
        for r in range(NT):
            del st["ep"][(7, r)]

    with (
        tc.tile_pool(name="persist", bufs=1) as pp,
        tc.tile_pool(name="expp", bufs=3) as epool,
        tc.tile_pool(name="psum_st", bufs=2, space="PSUM") as stp,
        tc.tile_pool(name="psum_fx", bufs=4, space="PSUM") as flex,
        tc.tile_pool(name="small", bufs=2) as sp,
    ):
        st["qT"] = pp.tile([128, NJ, T], F16, tag="qT", name="qT")
        st["kT"] = pp.tile([128, NJ, T], F16, tag="kT", name="kT")
        st["v"] = pp.tile([128, NT, HPC, DH + 1], F16, tag="v", name="v")
        st["trim"] = pp.tile([128, 128], F16, tag="trim", name="trim")
        st["ident"] = pp.tile([128, 128], F16, tag="ident", name="ident")
        # normalized y staging: [pair parity, qt, head parity, dh]
        st["yn"] = pp.tile([128, 2, NT, 2, DH], F16, tag="yn", name="yn")
        st["epool"], st["stp"], st["flex"], st["sp"] = epool, stp, flex, sp
        st["ep"], st["yT"] = {}, {}

        warm = pp.tile([1, DH], F16, tag="warm", name="warm")
        nc.gpsimd.memset(warm[:], 1.0)
        nc.gpsimd.memset(st["v"][:, :, :, DH:DH + 1], 1.0)
        nc.scalar.activation(warm[:], warm[:], EXP, scale=1.0)

        # phase-A pools, closed by schedule markers (LIFO: wqk on top)
        xp_ctx = tc.tile_pool(name="x8pool", bufs=1)
        xp = xp_ctx.__enter__()
        wv_ctx = tc.tile_pool(name="wvpool", bufs=1)
        wv = wv_ctx.__enter__()
        wqk_ctx = tc.tile_pool(name="wqkpool", bufs=1)
        wqk = wqk_ctx.__enter__()

        st["x8h"] = xp.tile([128, 4, 2, T], F8, tag="x8h", name="x8h")
        st["x8l"] = xp.tile([128, 4, 2, T], F8, tag="x8l", name="x8l")
        st["w8"] = {nm: wqk.tile([128, 4, 2, DQ], F8, tag=nm, name=nm)
                    for nm in ("wqh", "wql", "wkh", "wkl")}
        st["w8v"] = {nm: wv.tile([128, 4, 2, DQ], F8, tag=nm, name=nm)
                     for nm in ("wvh", "wvl")}

        # DMA order: wq split across queues, then x8 c-descending on
        # both queues, wk, wv; trimask/ident late (needed ~20us in).
        nc.sync.dma_start(st["w8"]["wqh"][:], w8_d["wqh"])
        nc.gpsimd.dma_start(st["w8"]["wql"][:], w8_d["wql"])
        for c in range(3, -1, -1):
            nc.sync.dma_start(
                st["x8h"][:, :, :, 512 * c:512 * (c + 1)], x8h_d[c])
            nc.gpsimd.dma_start(
                st["x8l"][:, :, :, 512 * c:512 * (c + 1)], x8l_d[c])
        nc.sync.dma_start(st["w8"]["wkh"][:], w8_d["wkh"])
        nc.gpsimd.dma_start(st["w8"]["wkl"][:], w8_d["wkl"])
        nc.sync.dma_start(st["w8v"]["wvh"][:], w8_d["wvh"])
        nc.gpsimd.dma_start(st["w8v"]["wvl"][:], w8_d["wvl"])
        nc.sync.dma_start(st["trim"][:], tm_d)
        nc.sync.dma_start(st["ident"][:], id_d)

        late_ctxs = []
        for op in SCHEDULE:
            kind = op[0]
            if kind == "qk":
                emit_qk(op[1])
            elif kind == "s":
                emit_s(op[1], op[2], op[3])
            elif kind == "vp":
                emit_vp(op[1])
            elif kind == "pv":
                emit_pv(op[1])
            elif kind == "tp":
                emit_tp(op[1])
            elif kind == "tail":
                emit_tail()
            elif kind == "close_wqk":
                wqk_ctx.__exit__(None, None, None)
            elif kind == "close_x8":
                wv_ctx.__exit__(None, None, None)
                xp_ctx.__exit__(None, None, None)
            elif kind == "open_late":
                lp_ctx = tc.tile_pool(name="late", bufs=1)
                lp = lp_ctx.__enter__()
                ytp_ctx = tc.tile_pool(name="ytp", bufs=3)
                st["ytp"] = ytp_ctx.__enter__()
                late_ctxs = [ytp_ctx, lp_ctx]
                st["lp"] = lp
                st["woT"] = lp.tile([128, NJ, D], F16, tag="woT", name="woT")
                nc.sync.dma_start(
                    st["woT"][:], wo_d.rearrange("(j p) n -> p j n", p=128))
            else:
                raise ValueError(op)
        for ctx in late_ctxs:
            ctx.__exit__(None, None, None)


def build_nc():
    nc = bacc.Bacc("TRN2", target_bir_lowering=False, debug=False)
    with tile.TileContext(nc) as tc:
        _body(tc)
    nc.compile()
    return nc


_nc_cache = None


def _get_nc():
    global _nc_cache
    if _nc_cache is None:
        _nc_cache = build_nc()
    return _nc_cache


def make_in_maps(x, W_q, W_k, W_v, W_o):
    x = np.asarray(x, dtype=np.float32)
    W_q = np.asarray(W_q, dtype=np.float32)
    W_k = np.asarray(W_k, dtype=np.float32)
    W_v = np.asarray(W_v, dtype=np.float32)
    W_o = np.asarray(W_o, dtype=np.float32)
    in_maps = []
    for c in range(8):
        b, g = divmod(c, 2)
        sl = slice(DQ * g, DQ * (g + 1))
        im = {"trimask": TRIMASK, "ident": IDENT}
        xs = np.ascontiguousarray(x[b].T) * X_SCALE
        xh, xl = _split8(xs)
        for nm, a in (("x8h", xh), ("x8l", xl)):
            im[nm] = np.ascontiguousarray(
                a.reshape(4, 2, 128, 4, 512).transpose(3, 2, 0, 1, 4)
            )
        for nm, W in (("wq", W_q), ("wk", W_k), ("wv", W_v)):
            Ws = np.ascontiguousarray(W[sl].T) * W_SCALE
            wh, wl = _split8(Ws)
            for suf, a in (("h", wh), ("l", wl)):
                im[nm + suf] = np.ascontiguousarray(
                    a.reshape(4, 2, 128, DQ).transpose(2, 0, 1, 3)
                )
        im["wot"] = np.ascontiguousarray(W_o[:, sl].T).astype(np.float16)
        in_maps.append(im)
    return in_maps


def kernel(x, W_q, W_k, W_v, W_o, b_o):
    global LAST
    nc = _get_nc()
    in_maps = make_in_maps(x, W_q, W_k, W_v, W_o)
    res = bass_utils.run_bass_kernel_spmd(
        nc, in_maps, core_ids=list(range(8)), trace=TRACE
    )
    LAST = res
    parts = [np.asarray(res.results[c]["out"], dtype=np.float32) for c in range(8)]
    b_o = np.asarray(b_o, dtype=np.float32)
    out = np.stack([parts[2 * b] + parts[2 * b + 1] for b in range(4)])
    out *= OUT_DESCALE
    out += b_o[None, None, :]
    return out.astype(np.float32)



# revision 29
# speedup vs baseline: 1.0169x; 1.0169x over previous
"""Causal self-attention kernel for 8 Trainium2 NeuronCores.

Problem (hardcoded): x [4, 2048, 1024], torch-style Linear weights
W_q/W_k/W_v/W_o [1024, 1024], b_o [1024]; 16 heads, head_dim 64,
causal softmax attention, out = attn(x) @ W_o.T + b_o.

Sharding: 8 cores = 4 batches x 2 head-groups (8 heads each).
Each core computes a partial output  y_g @ W_o[:, g].T  for its batch;
the host sums the two head-group partials, rescales, and adds b_o.

Per-core pipeline:
  phase A: QKV projections in fp8e4m3 DoubleRow (2 contraction rows per
           partition, 0.5 PE cycles/row) with two-sided error
           compensation: x ~ x_hi + x_lo, W ~ W_hi + W_lo (all fp8,
           host-split, pre-scaled by 8/64 into e4m3 range), accumulate
           x_hi*W_hi + x_lo*W_hi + x_hi*W_lo in PSUM (12 matmuls per
           512-wide tile = 0.75x the fp16 cost).  q/k land as qT/kT
           [dq, T] fp16; v natural [tok, h, 65] fp16 with ones column,
           produced per head-pair so PV(0)/PV(1) unblock early (they
           gate the exp pipeline via ep-buffer recycling).
  phase B: per head: scores S^T[k, q] fp16 (contraction dh=64) ->
           exp on ACT (scale 2^-21 folds softmax scale and the fp8
           pre-scales) into fp16 ep tiles [keys, q-span], causal span
           exact at 128 granularity; diagonal blocks masked via
           triangular mask-multiply on Pool (SBUF only - Pool cannot
           touch PSUM on TRN2).
  phase C: PV with moving dim = dh: y[q, 65] = sum_r ep_r^T @ v_r
           (65 PE cycles per causal block); col 64 is the softmax
           denominator; normalize via DVE reciprocal + DVE
           tensor_scalar (per-partition scalar, extra 0.25 fold);
           head-pair transposes on PE into yT (PSUM->SBUF move by DMA).
  phase D: out accumulated in fp16 SBUF: stage A fuses yT_{0,1,2} @ Wo
           in one PSUM group after pair 2, stage B adds yT_3 @ Wo in a
           per-query-tile tail pipeline woven with PV(7)/transpose,
           DMA out fp16 on both queues (host rescales by 1/128).

Emission order is schedule-driven (SCHEDULE below) so per-engine
program order can be tuned against the cost model.
"""

import numpy as np
import ml_dtypes

import concourse.bass as bass
import concourse.tile as tile
import concourse.mybir as mybir
from concourse import bacc
from concourse import bass_utils

T = 2048
D = 1024
HPC = 8            # heads per core
DH = 64
DQ = HPC * DH      # 512, per-core projection width
NT = T // 128      # 16 row tiles
NJ = DQ // 128     # 4 dq tiles

F32 = mybir.dt.float32
F16 = mybir.dt.float16
F8 = mybir.dt.float8e4
EXP = mybir.ActivationFunctionType.Exp
DR = mybir.MatmulPerfMode.DoubleRow
MUL = mybir.AluOpType.mult
E4NP = ml_dtypes.float8_e4m3

X_SCALE = 8.0      # host pre-scale of x before fp8 split
W_SCALE = 64.0     # host pre-scale of W_{q,k,v} before fp8 split
EXP_SCALE = 0.125 / (X_SCALE * W_SCALE) ** 2   # == 2^-21, exact
Y_FOLD = 0.25                                  # extra fold in normalize
OUT_DESCALE = 1.0 / (X_SCALE * W_SCALE * Y_FOLD)   # host undo (1/128)

TRACE = False
LAST = None        # BassKernelResults of the most recent run

TRIMASK = np.triu(np.ones((128, 128), dtype=np.float16))
IDENT = np.eye(128, dtype=np.float16)

# ep tag bufs: big tiles (small r) need depth 3 so exp(h) is only gated
# by PV(h-3); late-emitted small tiles can ride at depth 2 (their gate,
# PV(h-2)-end, passes long before their emission slot).
EP_BUFS = {r: (3 if r < 8 else 2) for r in range(NT)}

# Emission order (per-engine program order).  Ops:
#   ("qk", j)         Q+K projection, dq-tile j (c-descending)
#   ("s", h)          scores + exp + diag mask, head h
#   ("vp", p)         V projection, head pair p (all 16 token tiles)
#   ("pv", h)         attention*V + normalize, head h
#   ("tp", j)         pair transpose into yT_j (j <= 2)
#   ("oja",)          out-proj stage A: yT_{0,1,2} fused PSUM groups
#   ("tail",)         woven PV(7)+transpose+out-proj stage B+DMA
#   ("close_wqk",) ("close_x8",) ("open_late",)   pool scope markers
SCHEDULE = [
    # qk(0) interleaved with s(0)'s first chunks: the exp stream on ACT
    # (the critical engine through ~80% of the kernel) starts ~6us
    # earlier than emitting qk(0) en bloc.
    ("qkg", 0, 0, 0), ("qkg", 0, 0, 1), ("qkg", 0, 1, 0),
    ("s1", 0, 0, 2),
    ("qkg", 0, 0, 2), ("s1", 0, 2, 4),
    ("qkg", 0, 1, 1), ("s1", 0, 4, 6),
    ("qkg", 0, 0, 3), ("s1", 0, 6, 8),
    ("qkg", 0, 1, 2), ("s2", 0, 0, 4),
    ("qkg", 0, 1, 3), ("s2", 0, 4, 8),
    ("s", 0, 8, NT),
    # s(1) (same qT/kT block as s(0)) interleaved through qk(1) so ACT
    # never drains dry between heads.
    ("qkg", 1, 0, 0), ("qkg", 1, 0, 1),
    ("s1", 1, 0, 2), ("qkg", 1, 1, 0),
    ("s1", 1, 2, 4), ("qkg", 1, 0, 2),
    ("s1", 1, 4, 6), ("qkg", 1, 1, 1),
    ("s1", 1, 6, 8), ("qkg", 1, 0, 3),
    ("s2", 1, 0, 4), ("qkg", 1, 1, 2),
    ("s2", 1, 4, 8), ("qkg", 1, 1, 3),
    ("s", 1, 8, NT),
    ("vp", 0), ("pv", 0),
    ("s1", 2, 0, 2), ("qkg", 2, 0, 0),
    ("s1", 2, 2, 4), ("qkg", 2, 0, 1),
    ("s1", 2, 4, 6), ("qkg", 2, 1, 0),
    ("s1", 2, 6, 8), ("qkg", 2, 0, 2),
    ("s2", 2, 0, 4), ("qkg", 2, 1, 1),
    ("s2", 2, 4, 8), ("qkg", 2, 0, 3),
    ("s", 2, 8, 12), ("qkg", 2, 1, 2),
    ("s", 2, 12, NT), ("qkg", 2, 1, 3),
    ("s1", 3, 0, 2), ("qkg", 3, 0, 0),
    ("s1", 3, 2, 4), ("qkg", 3, 0, 1),
    ("s1", 3, 4, 6), ("qkg", 3, 1, 0),
    ("s1", 3, 6, 8), ("qkg", 3, 0, 2),
    ("s2", 3, 0, 4), ("qkg", 3, 1, 1),
    ("s2", 3, 4, 8), ("qkg", 3, 0, 3),
    ("s", 3, 8, 12), ("qkg", 3, 1, 2),
    ("s", 3, 12, NT), ("qkg", 3, 1, 3),
    ("close_wqk",),
    # From here to the end every score chunk is woven with non-score PE
    # work at fine grain: the stt pool is 3-deep, so PE stalls whenever
    # it runs >3 score chunks ahead of ACT — fillers absorb that.  The
    # tail stages (pv7/tp3/oj) are themselves woven into s(7) so the
    # out-projection tracks exp(7,qt) instead of starting after it.
    ("vp", 1), ("pv", 1),
    ("s1", 4, 0, 2), ("vp", 2, 0, 4),
    ("s1", 4, 2, 4), ("vp", 2, 4, 8),
    ("s1", 4, 4, 6), ("vp", 2, 8, 12),
    ("s1", 4, 6, 8), ("vp", 2, 12, 16),
    ("s2", 4, 0, 4), ("pv", 2, 0, 5),
    ("s2", 4, 4, 8), ("pv", 2, 5, 10),
    ("s", 4, 8, 12), ("pv", 2, 10, 16),
    ("s", 4, 12, NT),
    ("s1", 5, 0, 2), ("pv", 3, 0, 4),
    ("s1", 5, 2, 4), ("pv", 3, 4, 8),
    ("s1", 5, 4, 6), ("pv", 3, 8, 12),
    ("s1", 5, 6, 8), ("pv", 3, 12, 16),
    ("open_ytp",),
    ("s2", 5, 0, 4), ("tp", 0, 0, 8),
    ("s2", 5, 4, 8), ("tp", 0, 8, 16),
    ("s", 5, 8, 12), ("pv", 4, 0, 4),
    ("s", 5, 12, NT), ("pv", 4, 4, 8),
    ("s1", 6, 0, 2), ("vp", 3, 0, 4),
    ("s1", 6, 2, 4), ("vp", 3, 4, 8),
    ("s1", 6, 4, 6), ("vp", 3, 8, 12),
    ("s1", 6, 6, 8), ("vp", 3, 12, 16),
    ("close_x8",), ("open_late",),
    ("s2", 6, 0, 4), ("tp", 1, 0, 8),
    ("s2", 6, 4, 8), ("tp", 1, 8, 16),
    ("s", 6, 8, 12), ("pv", 4, 8, 16),
    ("s", 6, 12, NT), ("pv", 5, 0, 6),
    ("s1", 7, 0, 2), ("pv", 5, 6, 11),
    ("s1", 7, 2, 4), ("pv", 5, 11, 16),
    ("s1", 7, 4, 6), ("tp", 2, 0, 8),
    ("s1", 7, 6, 8), ("tp", 2, 8, 16),
    ("s2", 7, 0, 4), ("pv", 6, 0, 6),
    ("s2", 7, 4, 8), ("pv", 6, 6, 11),
    ("s", 7, 8, 10), ("pv", 6, 11, 16),
    ("s", 7, 10, 12), ("tail", 0, 3),
    ("s", 7, 12, 14), ("tail", 3, 6),
    ("s", 7, 14, NT), ("tail", 6, NT),
]


def _split8(a):
    """Two-sided fp8 split: a ~ hi + lo, both e4m3."""
    hi = np.clip(a, -240, 240).astype(E4NP)
    lo = np.clip(a - hi.astype(np.float32), -240, 240).astype(E4NP)
    return hi, lo


def _body(tc):
    nc = tc.nc
    # fp8 x: [c, p, s, i, col]: row 256s+128i+p of x.T*8, col 512c+col
    x8h_d = nc.dram_tensor("x8h", (4, 128, 4, 2, 512), F8, kind="ExternalInput").ap()
    x8l_d = nc.dram_tensor("x8l", (4, 128, 4, 2, 512), F8, kind="ExternalInput").ap()
    w8_d = {}
    for nm in ("wqh", "wql", "wkh", "wkl", "wvh", "wvl"):
        w8_d[nm] = nc.dram_tensor(nm, (128, 4, 2, DQ), F8, kind="ExternalInput").ap()
    wo_d = nc.dram_tensor("wot", (DQ, D), F16, kind="ExternalInput").ap()
    tm_d = nc.dram_tensor("trimask", (128, 128), F16, kind="ExternalInput").ap()
    id_d = nc.dram_tensor("ident", (128, 128), F16, kind="ExternalInput").ap()
    out_d = nc.dram_tensor("out", (T, D), F16, kind="ExternalOutput").ap()

    st = {}  # emission state

    def emit_qkg(j, qk, c):
        # One projection psum group: dq-block j, Q (qk=0) or K (qk=1),
        # token chunk c.
        x8h, x8l, w8 = st["x8h"], st["x8l"], st["w8"]
        if qk == 0:
            dest, wh, wl = st["qT"], w8["wqh"], w8["wql"]
        else:
            dest, wh, wl = st["kT"], w8["wkh"], w8["wkl"]
        terms = ((x8h, wh), (x8l, wh), (x8h, wl))
        ps = st["flex"].tile([128, 512], F32, tag="fx", name=f"p{j}_{c}")
        ki = 0
        for (xs, ws) in terms:
            for s in range(4):
                nc.tensor.matmul(
                    ps[:],
                    ws[:, s, :, 128 * j:128 * (j + 1)],
                    xs[:, s, :, 512 * c:512 * (c + 1)],
                    start=(ki == 0), stop=(ki == 11),
                    perf_mode=DR,
                )
                ki += 1
        nc.vector.tensor_copy(dest[:, j, 512 * c:512 * (c + 1)], ps[:])

    def emit_qk(j):
        # c-ascending with K's early blocks hoisted: scores consume
        # qT (all c) and kT key-block r, so Q chunks and low-c K chunks
        # unblock score tiles soonest.
        for qk, c in ((0, 0), (0, 1), (1, 0), (0, 2),
                      (1, 1), (0, 3), (1, 2), (1, 3)):
            emit_qkg(j, qk, c)

    def emit_vp(p, t0=0, t1=NT):
        # V projection for head pair p: out columns [128p, 128p+128)
        x8h, x8l, w8v = st["x8h"], st["x8l"], st["w8v"]
        terms = ((x8h, w8v["wvh"]), (x8l, w8v["wvh"]), (x8h, w8v["wvl"]))
        for t in range(t0, t1):
            ps = st["flex"].tile([128, 128], F32, tag="fx", name=f"pv{p}_{t}")
            ki = 0
            for (xs, ws) in terms:
                for s in range(4):
                    nc.tensor.matmul(
                        ps[:],
                        xs[:, s, :, 128 * t:128 * (t + 1)],
                        ws[:, s, :, 128 * p:128 * (p + 1)],
                        start=(ki == 0), stop=(ki == 11),
                        perf_mode=DR,
                    )
                    ki += 1
            nc.vector.tensor_copy(
                st["v"][:, t, 2 * p:2 * p + 2, 0:DH],
                ps[:].rearrange("p (h d) -> p h d", h=2),
            )

    def emit_s_chunk(h, r, chunk):
        # chunk 0: q-span (s0, 1536) [or (s0, 2048) for r >= 4] + diag
        # mask; chunk 1: q-span (1536, 2048) for r < 4.  1536-wide stt
        # slots (3 PSUM banks x 2 bufs) give 160 activations instead of
        # 192 (each ACT instruction pays ~143ns of PSUM access) at the
        # same PE-ahead-of-ACT lookahead of 3072 score columns.
        qT, kT, trim = st["qT"], st["kT"], st["trim"]
        hp = (h % 2) * DH
        hj = h // 2
        s0 = 128 * r
        if chunk == 0:
            ep = st["epool"].tile([128, T - s0], F16, tag=f"e{r}",
                                  name=f"e{r}_h{h}", bufs=EP_BUFS[r])
            st["ep"][(h, r)] = ep
            lo, hi = s0, (1024 if r < 8 else 2048)
        else:
            ep = st["ep"][(h, r)]
            lo, hi = 1024, 2048
        stt = st["stp"].tile([128, hi - lo], F32, tag="st",
                             padded_shape=[128, 1024])
        pieces = ([(lo, 512), (512, hi)] if (h, r, chunk) == (0, 0, 0)
                  else [(lo, hi)])
        for (alo, ahi) in pieces:
            for n0 in range(alo, ahi, 512):
                n1 = min(n0 + 512, ahi)
                nc.tensor.matmul(
                    stt[:, n0 - lo:n1 - lo],
                    kT[hp:hp + DH, hj, s0:s0 + 128],
                    qT[hp:hp + DH, hj, n0:n1],
                    start=True, stop=True,
                )
            nc.scalar.activation(
                ep[:, alo - s0:ahi - s0], stt[:, alo - lo:ahi - lo],
                EXP, scale=EXP_SCALE,
            )
        if chunk == 0:
            nc.gpsimd.tensor_mul(ep[:, 0:128], ep[:, 0:128], trim[:])

    def emit_s(h, r0=0, r1=NT):
        for r in range(r0, r1):
            emit_s_chunk(h, r, 0)
            if r < 8:
                emit_s_chunk(h, r, 1)

    def pv_tile(h, qt):
        """One PV psum group + normalize for (head, query tile)."""
        v, yn = st["v"], st["yn"]
        par = (h // 2) % 2
        hq = h % 2
        ya = st["flex"].tile([128, DH + 1], F32, tag="fx", name=f"y{h}_{qt}")
        for r in range(qt + 1):
            ep = st["ep"][(h, r)]
            off = 128 * (qt - r)
            nc.tensor.matmul(
                ya[:], ep[:, off:off + 128], v[:, r, h, :],
                start=(r == 0), stop=(r == qt),
            )
        rec = st["sp"].tile([128, 1], F32, tag="rec", name=f"rec{h}_{qt}")
        nc.vector.reciprocal(rec[:], ya[:, DH:DH + 1])
        nc.vector.tensor_scalar(
            yn[:, par, qt, hq, :], ya[:, 0:DH], rec[:], Y_FOLD, MUL, MUL,
        )

    def emit_pv(h, qt0=0, qt1=NT):
        for qt in range(qt0, qt1):
            pv_tile(h, qt)
        if qt1 == NT:
            for r in range(NT):
                del st["ep"][(h, r)]

    def tp_tile(j, qt, yTj):
        tp = st["flex"].tile([128, 128], F16, tag="fx", name=f"tp{j}_{qt}")
        nc.tensor.transpose(tp[:], st["yn"][:, j % 2, qt, :, :], st["ident"][:])
        nc.vector.tensor_copy(yTj[:, 128 * qt:128 * (qt + 1)], tp[:])

    def emit_tp(j, q0=0, q1=NT):
        if q0 == 0:
            st["yT"][j] = st["ytp"].tile([128, T], F16, tag="yT",
                                         name=f"yT{j}")
        yTj = st["yT"][j]
        for qt in range(q0, q1):
            tp_tile(j, qt, yTj)

    def emit_tail(qt0=0, qt1=NT):
        # Software-pipelined tail, lag 2: stage 1 at qt = PV(7, qt) +
        # pair-3 transpose into yT3[qt]; stage 2 at qt-2 = the WHOLE
        # out projection (one 8-matmul psum group per qt) -> psum->sbuf
        # copies on ACT/DVE -> DMA out.  Range form so the schedule can
        # weave tail stages between s(7) chunks: oj(qt) starts as soon
        # as exp(7, qt) lands instead of after the whole s(7) block.
        woT = st["woT"]
        yT3s = st.setdefault("yT3s", {})

        def oj_qt(q):
            # Both 512-halves accumulate in one 2-bank stp tile (stp is
            # score-free by the tail); the PSUM->SBUF copies split across
            # ACT (idle after the last exp) and DVE so neither serializes
            # the per-qt chain.
            ps = st["stp"].tile([128, 1024], F32, tag="st", name=f"ob{q}")
            for d in range(2):
                for j in range(4):
                    lhs = (st["yT"][j][:, 128 * q:128 * (q + 1)]
                           if j < 3 else yT3s[q][:])
                    nc.tensor.matmul(
                        ps[:, 512 * d:512 * (d + 1)], lhs,
                        woT[:, j, 512 * d:512 * (d + 1)],
                        start=(j == 0), stop=(j == 3),
                    )
                # d=0 copy on ACT (mostly idle by the tail) overlaps the
                # d=1 matmuls; d=1 copy on DVE so neither serializes.
                ob = st["lp"].tile([128, 512], F16, tag="ob",
                                   name=f"ob{q}_{d}", bufs=4)
                if d == 0:
                    nc.scalar.copy(ob[:], ps[:, 0:512])
                else:
                    nc.vector.tensor_copy(ob[:], ps[:, 512:1024])
                qu = nc.sync if d == 0 else nc.gpsimd
                orow = out_d[128 * q:128 * (q + 1), 512 * d:512 * (d + 1)]
                if q == NT - 1:
                    # Final group: halve the last transfers so the
                    # end-of-kernel DMA+sem chain is shorter.
                    qu.dma_start(orow[:, 0:256], ob[:, 0:256])
                    qu2 = nc.gpsimd if d == 0 else nc.sync
                    qu2.dma_start(orow[:, 256:512], ob[:, 256:512])
                else:
                    qu.dma_start(orow, ob[:])
            del yT3s[q]

        for qt in range(qt0, qt1):
            pv_tile(7, qt)
            yT3s[qt] = st["lp"].tile([128, 128], F16, tag="yt3",
                                     name=f"yt3_{qt}", bufs=4)
            tp = st["flex"].tile([128, 128], F16, tag="fx", name=f"tp3_{qt}")
            nc.tensor.transpose(tp[:], st["yn"][:, 1, qt, :, :], st["ident"][:])
            nc.vector.tensor_copy(yT3s[qt][:], tp[:])
            if qt >= 2:
                oj_qt(qt - 2)
        if qt1 == NT:
            oj_qt(NT - 2)
            oj_qt(NT - 1)
            for r in range(NT):
                del st["ep"][(7, r)]

    with (
        tc.tile_pool(name="persist", bufs=1) as pp,
        tc.tile_pool(name="expp", bufs=3) as epool,
        tc.tile_pool(name="psum_st", bufs=3, space="PSUM") as stp,
        tc.tile_pool(name="psum_fx", bufs=2, space="PSUM") as flex,
        tc.tile_pool(name="small", bufs=2) as sp,
    ):
        st["qT"] = pp.tile([128, NJ, T], F16, tag="qT", name="qT")
        st["kT"] = pp.tile([128, NJ, T], F16, tag="kT", name="kT")
        st["v"] = pp.tile([128, NT, HPC, DH + 1], F16, tag="v", name="v")
        st["trim"] = pp.tile([128, 128], F16, tag="trim", name="trim")
        st["ident"] = pp.tile([128, 128], F16, tag="ident", name="ident")
        # normalized y staging: [pair parity, qt, head parity, dh]
        st["yn"] = pp.tile([128, 2, NT, 2, DH], F16, tag="yn", name="yn")
        st["epool"], st["stp"], st["flex"], st["sp"] = epool, stp, flex, sp
        st["ep"], st["yT"] = {}, {}

        warm = pp.tile([1, DH], F16, tag="warm", name="warm")
        nc.gpsimd.memset(warm[:], 1.0)
        nc.gpsimd.memset(st["v"][:, :, :, DH:DH + 1], 1.0)
        nc.scalar.activation(warm[:], warm[:], EXP, scale=1.0)

        # phase-A pools, closed by schedule markers (LIFO: wqk on top)
        xp_ctx = tc.tile_pool(name="x8pool", bufs=1)
        xp = xp_ctx.__enter__()
        wv_ctx = tc.tile_pool(name="wvpool", bufs=1)
        wv = wv_ctx.__enter__()
        wqk_ctx = tc.tile_pool(name="wqkpool", bufs=1)
        wqk = wqk_ctx.__enter__()

        st["x8h"] = xp.tile([128, 4, 2, T], F8, tag="x8h", name="x8h")
        st["x8l"] = xp.tile([128, 4, 2, T], F8, tag="x8l", name="x8l")
        st["w8"] = {nm: wqk.tile([128, 4, 2, DQ], F8, tag=nm, name=nm)
                    for nm in ("wqh", "wql", "wkh", "wkl")}
        st["w8v"] = {nm: wv.tile([128, 4, 2, DQ], F8, tag=nm, name=nm)
                     for nm in ("wvh", "wvl")}

        # DMA order: wq first, then x8 c-ascending with wk hoisted after
        # c1 (K_c0 runs ~4 groups in), wv/trim/ident late.  Full-tensor
        # transfers: each dma_start costs ~565ns of SP sequencer, so
        # finer slicing delays later chunks more than it helps the first.
        nc.sync.dma_start(st["w8"]["wqh"][:], w8_d["wqh"])
        nc.gpsimd.dma_start(st["w8"]["wql"][:], w8_d["wql"])
        for c in range(2):
            nc.sync.dma_start(
                st["x8h"][:, :, :, 512 * c:512 * (c + 1)], x8h_d[c])
            nc.gpsimd.dma_start(
                st["x8l"][:, :, :, 512 * c:512 * (c + 1)], x8l_d[c])
        nc.sync.dma_start(st["w8"]["wkh"][:], w8_d["wkh"])
        nc.gpsimd.dma_start(st["w8"]["wkl"][:], w8_d["wkl"])
        for c in range(2, 4):
            nc.sync.dma_start(
                st["x8h"][:, :, :, 512 * c:512 * (c + 1)], x8h_d[c])
            nc.gpsimd.dma_start(
                st["x8l"][:, :, :, 512 * c:512 * (c + 1)], x8l_d[c])
        nc.gpsimd.dma_start(st["trim"][:], tm_d)
        nc.sync.dma_start(st["w8v"]["wvh"][:], w8_d["wvh"])
        nc.gpsimd.dma_start(st["w8v"]["wvl"][:], w8_d["wvl"])
        nc.sync.dma_start(st["ident"][:], id_d)

        late_ctxs = []
        for op in SCHEDULE:
            kind = op[0]
            if kind == "qk":
                emit_qk(op[1])
            elif kind == "qkg":
                emit_qkg(op[1], op[2], op[3])
            elif kind == "s1":
                for r in range(op[2], op[3]):
                    emit_s_chunk(op[1], r, 0)
            elif kind == "s2":
                for r in range(op[2], op[3]):
                    emit_s_chunk(op[1], r, 1)
            elif kind == "s":
                emit_s(op[1], op[2], op[3])
            elif kind == "vp":
                emit_vp(op[1], *op[2:])
            elif kind == "pv":
                emit_pv(op[1], *op[2:])
            elif kind == "tp":
                emit_tp(op[1], *op[2:])
            elif kind == "tail":
                emit_tail(*op[1:])
            elif kind == "close_wqk":
                wqk_ctx.__exit__(None, None, None)
            elif kind == "close_x8":
                wv_ctx.__exit__(None, None, None)
                xp_ctx.__exit__(None, None, None)
            elif kind == "open_ytp":
                # Right-side pool: outlives the left-side x8/wv pools
                # that close later (close_x8), dodging LIFO ordering.
                ytp_ctx = tc.tile_pool(name="ytp", bufs=3, side="right")
                st["ytp"] = ytp_ctx.__enter__()
                late_ctxs.append(ytp_ctx)
            elif kind == "open_late":
                lp_ctx = tc.tile_pool(name="late", bufs=1)
                lp = lp_ctx.__enter__()
                late_ctxs.append(lp_ctx)
                st["lp"] = lp
                st["woT"] = lp.tile([128, NJ, D], F16, tag="woT", name="woT")
                nc.sync.dma_start(
                    st["woT"][:], wo_d.rearrange("(j p) n -> p j n", p=128))
            else:
                raise ValueError(op)
        for ctx in reversed(late_ctxs):
            ctx.__exit__(None, None, None)


def build_nc():
    nc = bacc.Bacc("TRN2", target_bir_lowering=False, debug=False)
    with tile.TileContext(nc) as tc:
        _body(tc)
    nc.compile()
    return nc


_nc_cache = None


def _get_nc():
    global _nc_cache
    if _nc_cache is None:
        _nc_cache = build_nc()
    return _nc_cache


def make_in_maps(x, W_q, W_k, W_v, W_o):
    x = np.asarray(x, dtype=np.float32)
    W_q = np.asarray(W_q, dtype=np.float32)
    W_k = np.asarray(W_k, dtype=np.float32)
    W_v = np.asarray(W_v, dtype=np.float32)
    W_o = np.asarray(W_o, dtype=np.float32)
    in_maps = []
    for c in range(8):
        b, g = divmod(c, 2)
        sl = slice(DQ * g, DQ * (g + 1))
        im = {"trimask": TRIMASK, "ident": IDENT}
        xs = np.ascontiguousarray(x[b].T) * X_SCALE
        xh, xl = _split8(xs)
        for nm, a in (("x8h", xh), ("x8l", xl)):
            im[nm] = np.ascontiguousarray(
                a.reshape(4, 2, 128, 4, 512).transpose(3, 2, 0, 1, 4)
            )
        for nm, W in (("wq", W_q), ("wk", W_k), ("wv", W_v)):
            Ws = np.ascontiguousarray(W[sl].T) * W_SCALE
            wh, wl = _split8(Ws)
            for suf, a in (("h", wh), ("l", wl)):
                im[nm + suf] = np.ascontiguousarray(
                    a.reshape(4, 2, 128, DQ).transpose(2, 0, 1, 3)
                )
        im["wot"] = np.ascontiguousarray(W_o[:, sl].T).astype(np.float16)
        in_maps.append(im)
    return in_maps


def kernel(x, W_q, W_k, W_v, W_o, b_o):
    global LAST
    nc = _get_nc()
    in_maps = make_in_maps(x, W_q, W_k, W_v, W_o)
    res = bass_utils.run_bass_kernel_spmd(
        nc, in_maps, core_ids=list(range(8)), trace=TRACE
    )
    LAST = res
    parts = [np.asarray(res.results[c]["out"], dtype=np.float32) for c in range(8)]
    b_o = np.asarray(b_o, dtype=np.float32)
    out = np.stack([parts[2 * b] + parts[2 * b + 1] for b in range(4)])
    out *= OUT_DESCALE
    out += b_o[None, None, :]
    return out.astype(np.float32)



# revision 34
# speedup vs baseline: 1.0279x; 1.0109x over previous
"""Causal self-attention kernel for 8 Trainium2 NeuronCores.

Problem (hardcoded): x [4, 2048, 1024], torch-style Linear weights
W_q/W_k/W_v/W_o [1024, 1024], b_o [1024]; 16 heads, head_dim 64,
causal softmax attention, out = attn(x) @ W_o.T + b_o.

Sharding: 8 cores = 4 batches x 2 head-groups (8 heads each).
Each core computes a partial output  y_g @ W_o[:, g].T  for its batch;
the host sums the two head-group partials, rescales, and adds b_o.

Per-core pipeline:
  phase A: QKV projections in fp8e4m3 DoubleRow (2 contraction rows per
           partition, 0.5 PE cycles/row) with two-sided error
           compensation: x ~ x_hi + x_lo, W ~ W_hi + W_lo (all fp8,
           host-split, pre-scaled by 8/64 into e4m3 range), accumulate
           x_hi*W_hi + x_lo*W_hi + x_hi*W_lo in PSUM (12 matmuls per
           512-wide tile = 0.75x the fp16 cost).  q/k land as qT/kT
           [dq, T] fp16; v natural [tok, h, 65] fp16 with ones column,
           produced per head-pair so PV(0)/PV(1) unblock early (they
           gate the exp pipeline via ep-buffer recycling).
  phase B: per head: scores S^T[k, q] fp16 (contraction dh=64) ->
           exp on ACT (scale 2^-21 folds softmax scale and the fp8
           pre-scales) into fp16 ep tiles [keys, q-span], causal span
           exact at 128 granularity; diagonal blocks masked via
           triangular mask-multiply on Pool (SBUF only - Pool cannot
           touch PSUM on TRN2).
  phase C: PV with moving dim = dh: y[q, 65] = sum_r ep_r^T @ v_r
           (65 PE cycles per causal block); col 64 is the softmax
           denominator; normalize via DVE reciprocal + DVE
           tensor_scalar (per-partition scalar, extra 0.25 fold);
           head-pair transposes on PE into yT (PSUM->SBUF move by DMA).
  phase D: out accumulated in fp16 SBUF: stage A fuses yT_{0,1,2} @ Wo
           in one PSUM group after pair 2, stage B adds yT_3 @ Wo in a
           per-query-tile tail pipeline woven with PV(7)/transpose,
           DMA out fp16 on both queues (host rescales by 1/128).

Emission order is schedule-driven (SCHEDULE below) so per-engine
program order can be tuned against the cost model.
"""

import numpy as np
import ml_dtypes

import concourse.bass as bass
import concourse.tile as tile
import concourse.mybir as mybir
from concourse import bacc
from concourse import bass_utils

T = 2048
D = 1024
HPC = 8            # heads per core
DH = 64
DQ = HPC * DH      # 512, per-core projection width
NT = T // 128      # 16 row tiles
NJ = DQ // 128     # 4 dq tiles

F32 = mybir.dt.float32
F16 = mybir.dt.float16
F8 = mybir.dt.float8e4
EXP = mybir.ActivationFunctionType.Exp
DR = mybir.MatmulPerfMode.DoubleRow
MUL = mybir.AluOpType.mult
E4NP = ml_dtypes.float8_e4m3

X_SCALE = 8.0      # host pre-scale of x before fp8 split
W_SCALE = 64.0     # host pre-scale of W_{q,k} before fp8 split
W_SCALE_V = 16.0   # W_v pre-scale: folds the old 0.25 normalize factor
EXP_SCALE = 0.125 / (X_SCALE * W_SCALE) ** 2   # == 2^-21, exact
OUT_DESCALE = 1.0 / (X_SCALE * W_SCALE_V)      # host undo (1/128)

TRACE = False
LAST = None        # BassKernelResults of the most recent run

TRIMASK = np.triu(np.ones((128, 128), dtype=np.float16))
IDENT = np.eye(128, dtype=np.float16)

# ep tag bufs: big tiles (small r) need depth 3 so exp(h) is only gated
# by PV(h-3); late-emitted small tiles can ride at depth 2 (their gate,
# PV(h-2)-end, passes long before their emission slot).
EP_BUFS = {r: (3 if r < 8 else 2) for r in range(NT)}

# Emission order (per-engine program order).  Ops:
#   ("qk", j)         Q+K projection, dq-tile j (c-descending)
#   ("s", h)          scores + exp + diag mask, head h
#   ("vp", p)         V projection, head pair p (all 16 token tiles)
#   ("pv", h)         attention*V + normalize, head h
#   ("tp", j)         pair transpose into yT_j (j <= 2)
#   ("oja",)          out-proj stage A: yT_{0,1,2} fused PSUM groups
#   ("tail",)         woven PV(7)+transpose+out-proj stage B+DMA
#   ("close_wqk",) ("close_x8",) ("open_late",)   pool scope markers
SCHEDULE = [
    # qk(0) interleaved with s(0)'s first chunks: the exp stream on ACT
    # (the critical engine through ~80% of the kernel) starts ~6us
    # earlier than emitting qk(0) en bloc.
    ("qkg", 0, 0, 0), ("qkg", 0, 0, 1), ("qkg", 0, 1, 0),
    ("s1", 0, 0, 2),
    ("qkg", 0, 0, 2), ("s1", 0, 2, 4),
    ("qkg", 0, 1, 1), ("s1", 0, 4, 6),
    ("qkg", 0, 0, 3), ("s1", 0, 6, 8),
    ("qkg", 0, 1, 2), ("s2", 0, 0, 4),
    ("qkg", 0, 1, 3), ("s2", 0, 4, 8),
    ("s", 0, 8, NT),
    # s(1) (same qT/kT block as s(0)) interleaved through qk(1) so ACT
    # never drains dry between heads.
    ("qkg", 1, 0, 0), ("qkg", 1, 0, 1),
    ("s1", 1, 0, 2), ("qkg", 1, 1, 0),
    ("s1", 1, 2, 4), ("qkg", 1, 0, 2),
    ("s1", 1, 4, 6), ("qkg", 1, 1, 1),
    ("s1", 1, 6, 8), ("qkg", 1, 0, 3),
    ("s2", 1, 0, 4), ("qkg", 1, 1, 2),
    ("s2", 1, 4, 8), ("qkg", 1, 1, 3),
    ("s", 1, 8, NT),
    ("vp", 0), ("pv", 0),
    ("s1", 2, 0, 2), ("qkg", 2, 0, 0),
    ("s1", 2, 2, 4), ("qkg", 2, 0, 1),
    ("s1", 2, 4, 6), ("qkg", 2, 1, 0),
    ("s1", 2, 6, 8), ("qkg", 2, 0, 2),
    ("s2", 2, 0, 4), ("qkg", 2, 1, 1),
    ("s2", 2, 4, 8), ("qkg", 2, 0, 3),
    ("s", 2, 8, 12), ("qkg", 2, 1, 2),
    ("s", 2, 12, NT), ("qkg", 2, 1, 3),
    ("s1", 3, 0, 2), ("qkg", 3, 0, 0),
    ("s1", 3, 2, 4), ("qkg", 3, 0, 1),
    ("s1", 3, 4, 6), ("qkg", 3, 1, 0),
    ("s1", 3, 6, 8), ("qkg", 3, 0, 2),
    ("s2", 3, 0, 4), ("qkg", 3, 1, 1),
    ("s2", 3, 4, 8), ("qkg", 3, 0, 3),
    ("s", 3, 8, 12), ("qkg", 3, 1, 2),
    ("s", 3, 12, NT), ("qkg", 3, 1, 3),
    ("close_wqk",),
    # From here to the end every score chunk is woven with non-score PE
    # work at fine grain: the stt pool is 3-deep, so PE stalls whenever
    # it runs >3 score chunks ahead of ACT — fillers absorb that.  The
    # tail stages (pv7/tp3/oj) are themselves woven into s(7) so the
    # out-projection tracks exp(7,qt) instead of starting after it.
    ("vp", 1), ("pv", 1),
    ("s1", 4, 0, 2), ("vp", 2, 0, 4),
    ("s1", 4, 2, 4), ("vp", 2, 4, 8),
    ("s1", 4, 4, 6), ("vp", 2, 8, 12),
    ("s1", 4, 6, 8), ("vp", 2, 12, 16),
    ("s2", 4, 0, 4), ("pv", 2, 0, 4),
    ("s2", 4, 4, 8), ("pv", 2, 4, 8),
    ("s", 4, 8, 12), ("pv", 2, 8, 16),
    ("s", 4, 12, NT),
    ("s1", 5, 0, 2), ("pv", 3, 0, 4),
    ("s1", 5, 2, 4), ("pv", 3, 4, 8),
    ("s1", 5, 4, 6), ("pv", 3, 8, 12),
    ("s1", 5, 6, 8), ("pv", 3, 12, 16),
    ("open_ytp",),
    ("s2", 5, 0, 4), ("tp", 0, 0, 8),
    ("s2", 5, 4, 8), ("tp", 0, 8, 16),
    ("s", 5, 8, 12), ("pv", 4, 0, 4),
    ("s", 5, 12, NT), ("pv", 4, 4, 8),
    ("s1", 6, 0, 2), ("vp", 3, 0, 4),
    ("s1", 6, 2, 4), ("vp", 3, 4, 8),
    ("s1", 6, 4, 6), ("vp", 3, 8, 12),
    ("s1", 6, 6, 8), ("vp", 3, 12, 16),
    ("close_x8",), ("open_late",),
    ("s2", 6, 0, 4), ("tp", 1, 0, 8),
    ("s2", 6, 4, 8), ("tp", 1, 8, 16),
    ("s", 6, 8, 12), ("pv", 4, 8, 16),
    ("s", 6, 12, NT), ("pv", 5, 0, 4),
    ("s1", 7, 0, 2), ("pv", 5, 4, 8),
    ("s1", 7, 2, 4), ("pv", 5, 8, 16),
    ("s1", 7, 4, 6), ("tp", 2, 0, 8),
    ("s1", 7, 6, 8), ("tp", 2, 8, 16),
    ("s2", 7, 0, 4), ("pv", 6, 0, 4),
    ("s2", 7, 4, 8), ("pv", 6, 4, 8),
    ("s", 7, 8, 10), ("pv", 6, 8, 16),
    ("s", 7, 10, 12), ("tail", 0, 3),
    ("s", 7, 12, 14), ("tail", 3, 6),
    ("s", 7, 14, NT), ("tail", 6, NT),
]


def _split8(a):
    """Two-sided fp8 split: a ~ hi + lo, both e4m3."""
    hi = np.clip(a, -240, 240).astype(E4NP)
    lo = np.clip(a - hi.astype(np.float32), -240, 240).astype(E4NP)
    return hi, lo


def _body(tc):
    nc = tc.nc
    # fp8 x: [c, p, s, i, col]: row 256s+128i+p of x.T*8, col 512c+col
    x8h_d = nc.dram_tensor("x8h", (4, 128, 4, 2, 512), F8, kind="ExternalInput").ap()
    x8l_d = nc.dram_tensor("x8l", (4, 128, 4, 2, 512), F8, kind="ExternalInput").ap()
    w8_d = {}
    for nm in ("wqh", "wql", "wkh", "wkl", "wvh", "wvl"):
        w8_d[nm] = nc.dram_tensor(nm, (128, 4, 2, DQ), F8, kind="ExternalInput").ap()
    wo_d = nc.dram_tensor("wot", (DQ, D), F16, kind="ExternalInput").ap()
    tm_d = nc.dram_tensor("trimask", (128, 128), F16, kind="ExternalInput").ap()
    id_d = nc.dram_tensor("ident", (128, 128), F16, kind="ExternalInput").ap()
    out_d = nc.dram_tensor("out", (T, D), F16, kind="ExternalOutput").ap()

    st = {}  # emission state

    def emit_qkg(j, qk, c):
        # One projection psum group: dq-block j, Q (qk=0) or K (qk=1),
        # token chunk c.
        x8h, x8l, w8 = st["x8h"], st["x8l"], st["w8"]
        if qk == 0:
            dest, wh, wl = st["qT"], w8["wqh"], w8["wql"]
        else:
            dest, wh, wl = st["kT"], w8["wkh"], w8["wkl"]
        terms = ((x8h, wh), (x8l, wh), (x8h, wl))
        ps = st["flex"].tile([128, 512], F32, tag="fx", name=f"p{j}_{c}")
        ki = 0
        for (xs, ws) in terms:
            for s in range(4):
                nc.tensor.matmul(
                    ps[:],
                    ws[:, s, :, 128 * j:128 * (j + 1)],
                    xs[:, s, :, 512 * c:512 * (c + 1)],
                    start=(ki == 0), stop=(ki == 11),
                    perf_mode=DR,
                )
                ki += 1
        nc.vector.tensor_copy(dest[:, j, 512 * c:512 * (c + 1)], ps[:])

    def emit_qk(j):
        # c-ascending with K's early blocks hoisted: scores consume
        # qT (all c) and kT key-block r, so Q chunks and low-c K chunks
        # unblock score tiles soonest.
        for qk, c in ((0, 0), (0, 1), (1, 0), (0, 2),
                      (1, 1), (0, 3), (1, 2), (1, 3)):
            emit_qkg(j, qk, c)

    def emit_vp(p, t0=0, t1=NT):
        # V projection for head pair p: out columns [128p, 128p+128)
        x8h, x8l, w8v = st["x8h"], st["x8l"], st["w8v"]
        terms = ((x8h, w8v["wvh"]), (x8l, w8v["wvh"]), (x8h, w8v["wvl"]))
        for t in range(t0, t1):
            ps = st["flex"].tile([128, 128], F32, tag="fx", name=f"pv{p}_{t}")
            ki = 0
            for (xs, ws) in terms:
                for s in range(4):
                    nc.tensor.matmul(
                        ps[:],
                        xs[:, s, :, 128 * t:128 * (t + 1)],
                        ws[:, s, :, 128 * p:128 * (p + 1)],
                        start=(ki == 0), stop=(ki == 11),
                        perf_mode=DR,
                    )
                    ki += 1
            nc.vector.tensor_copy(
                st["v"][:, t, 2 * p:2 * p + 2, 0:DH],
                ps[:].rearrange("p (h d) -> p h d", h=2),
            )

    def emit_s_chunk(h, r, chunk):
        # chunk 0: q-span (s0, 1536) [or (s0, 2048) for r >= 4] + diag
        # mask; chunk 1: q-span (1536, 2048) for r < 4.  1536-wide stt
        # slots (3 PSUM banks x 2 bufs) give 160 activations instead of
        # 192 (each ACT instruction pays ~143ns of PSUM access) at the
        # same PE-ahead-of-ACT lookahead of 3072 score columns.
        qT, kT, trim = st["qT"], st["kT"], st["trim"]
        hp = (h % 2) * DH
        hj = h // 2
        s0 = 128 * r
        if chunk == 0:
            ep = st["epool"].tile([128, T - s0], F16, tag=f"e{r}",
                                  name=f"e{r}_h{h}", bufs=EP_BUFS[r])
            st["ep"][(h, r)] = ep
            lo, hi = s0, (1024 if r < 8 else 2048)
        else:
            ep = st["ep"][(h, r)]
            lo, hi = 1024, 2048
        stt = st["stp"].tile([128, hi - lo], F32, tag="st",
                             padded_shape=[128, 1024])
        pieces = ([(lo, 512), (512, hi)] if (h, r, chunk) == (0, 0, 0)
                  else [(lo, hi)])
        for (alo, ahi) in pieces:
            for n0 in range(alo, ahi, 512):
                n1 = min(n0 + 512, ahi)
                nc.tensor.matmul(
                    stt[:, n0 - lo:n1 - lo],
                    kT[hp:hp + DH, hj, s0:s0 + 128],
                    qT[hp:hp + DH, hj, n0:n1],
                    start=True, stop=True,
                )
            nc.scalar.activation(
                ep[:, alo - s0:ahi - s0], stt[:, alo - lo:ahi - lo],
                EXP, scale=EXP_SCALE,
            )
        if chunk == 0:
            nc.gpsimd.tensor_mul(ep[:, 0:128], ep[:, 0:128], trim[:])

    def emit_s(h, r0=0, r1=NT):
        for r in range(r0, r1):
            emit_s_chunk(h, r, 0)
            if r < 8:
                emit_s_chunk(h, r, 1)

    def pv_quad(h, q0, nq):
        """PV psum groups for nq query tiles sharing one PSUM tile, with
        a single strided reciprocal + broadcast-multiply normalize.  The
        0.25 fold lives in the host-side W_v scale, so the normalize is
        a pure multiply and the per-tile DVE chain (two ops per query
        tile) collapses to two ops per quad."""
        v, yn = st["v"], st["yn"]
        par = (h // 2) % 2
        hq = h % 2
        ya = st["flex"].tile([128, nq, DH + 1], F32, tag="fx",
                             name=f"y{h}_{q0}", padded_shape=[128, 4, DH + 1])
        for qi in range(nq):
            qt = q0 + qi
            for r in range(qt + 1):
                ep = st["ep"][(h, r)]
                off = 128 * (qt - r)
                nc.tensor.matmul(
                    ya[:, qi, :], ep[:, off:off + 128], v[:, r, h, :],
                    start=(r == 0), stop=(r == qt),
                )
        rec = st["sp"].tile([128, nq], F32, tag="rec", name=f"rec{h}_{q0}",
                            padded_shape=[128, 4])
        nc.vector.reciprocal(rec[:], ya[:, :, DH])
        rb = rec[:].unsqueeze(2).broadcast_to((128, nq, DH))
        nc.vector.tensor_tensor(
            yn[:, par, q0:q0 + nq, hq, :], ya[:, :, 0:DH], rb, MUL,
        )

    def emit_pv(h, qt0=0, qt1=NT):
        for q0 in range(qt0, qt1, 4):
            pv_quad(h, q0, min(4, qt1 - q0))
        if qt1 == NT:
            for r in range(NT):
                del st["ep"][(h, r)]

    def tp_tile(j, qt, yTj):
        tp = st["flex"].tile([128, 128], F16, tag="fx", name=f"tp{j}_{qt}")
        nc.tensor.transpose(tp[:], st["yn"][:, j % 2, qt, :, :], st["ident"][:])
        nc.vector.tensor_copy(yTj[:, 128 * qt:128 * (qt + 1)], tp[:])

    def emit_tp(j, q0=0, q1=NT):
        if q0 == 0:
            st["yT"][j] = st["ytp"].tile([128, T], F16, tag="yT",
                                         name=f"yT{j}")
        yTj = st["yT"][j]
        for qt in range(q0, q1):
            tp_tile(j, qt, yTj)

    def emit_tail(qt0=0, qt1=NT):
        # Software-pipelined tail, lag 2: stage 1 at qt = PV(7, qt) +
        # pair-3 transpose into yT3[qt]; stage 2 at qt-2 = the WHOLE
        # out projection (one 8-matmul psum group per qt) -> psum->sbuf
        # copies on ACT/DVE -> DMA out.  Range form so the schedule can
        # weave tail stages between s(7) chunks: oj(qt) starts as soon
        # as exp(7, qt) lands instead of after the whole s(7) block.
        woT = st["woT"]
        yT3s = st.setdefault("yT3s", {})

        def oj_qt(q):
            # Both 512-halves accumulate in one 2-bank stp tile (stp is
            # score-free by the tail); the PSUM->SBUF copies split across
            # ACT (idle after the last exp) and DVE so neither serializes
            # the per-qt chain.
            ps = st["stp"].tile([128, 1024], F32, tag="st", name=f"ob{q}")
            for d in range(2):
                for j in range(4):
                    lhs = (st["yT"][j][:, 128 * q:128 * (q + 1)]
                           if j < 3 else yT3s[q][:])
                    nc.tensor.matmul(
                        ps[:, 512 * d:512 * (d + 1)], lhs,
                        woT[:, j, 512 * d:512 * (d + 1)],
                        start=(j == 0), stop=(j == 3),
                    )
                # d=0 copy on ACT (mostly idle by the tail) overlaps the
                # d=1 matmuls; d=1 copy on DVE so neither serializes.
                ob = st["lp"].tile([128, 512], F16, tag="ob",
                                   name=f"ob{q}_{d}", bufs=4)
                if d == 0:
                    nc.scalar.copy(ob[:], ps[:, 0:512])
                else:
                    nc.vector.tensor_copy(ob[:], ps[:, 512:1024])
                qu = nc.sync if d == 0 else nc.gpsimd
                orow = out_d[128 * q:128 * (q + 1), 512 * d:512 * (d + 1)]
                if q == NT - 1:
                    # Final group: halve the last transfers so the
                    # end-of-kernel DMA+sem chain is shorter.
                    qu.dma_start(orow[:, 0:256], ob[:, 0:256])
                    qu2 = nc.gpsimd if d == 0 else nc.sync
                    qu2.dma_start(orow[:, 256:512], ob[:, 256:512])
                else:
                    qu.dma_start(orow, ob[:])
            del yT3s[q]

        for qt in range(qt0, qt1):
            pv_quad(7, qt, 1)
            yT3s[qt] = st["lp"].tile([128, 128], F16, tag="yt3",
                                     name=f"yt3_{qt}", bufs=4)
            tp = st["flex"].tile([128, 128], F16, tag="fx", name=f"tp3_{qt}")
            nc.tensor.transpose(tp[:], st["yn"][:, 1, qt, :, :], st["ident"][:])
            nc.vector.tensor_copy(yT3s[qt][:], tp[:])
            if qt >= 2:
                oj_qt(qt - 2)
        if qt1 == NT:
            oj_qt(NT - 2)
            oj_qt(NT - 1)
            for r in range(NT):
                del st["ep"][(7, r)]

    with (
        tc.tile_pool(name="persist", bufs=1) as pp,
        tc.tile_pool(name="expp", bufs=3) as epool,
        tc.tile_pool(name="psum_st", bufs=3, space="PSUM") as stp,
        tc.tile_pool(name="psum_fx", bufs=2, space="PSUM") as flex,
        tc.tile_pool(name="small", bufs=4) as sp,
    ):
        st["qT"] = pp.tile([128, NJ, T], F16, tag="qT", name="qT")
        st["kT"] = pp.tile([128, NJ, T], F16, tag="kT", name="kT")
        st["v"] = pp.tile([128, NT, HPC, DH + 1], F16, tag="v", name="v")
        st["trim"] = pp.tile([128, 128], F16, tag="trim", name="trim")
        st["ident"] = pp.tile([128, 128], F16, tag="ident", name="ident")
        # normalized y staging: [pair parity, qt, head parity, dh]
        st["yn"] = pp.tile([128, 2, NT, 2, DH], F16, tag="yn", name="yn")
        st["epool"], st["stp"], st["flex"], st["sp"] = epool, stp, flex, sp
        st["ep"], st["yT"] = {}, {}

        warm = pp.tile([1, DH], F16, tag="warm", name="warm")
        nc.gpsimd.memset(warm[:], 1.0)
        nc.gpsimd.memset(st["v"][:, :, :, DH:DH + 1], 1.0)
        nc.scalar.activation(warm[:], warm[:], EXP, scale=1.0)

        # phase-A pools, closed by schedule markers (LIFO: wqk on top)
        xp_ctx = tc.tile_pool(name="x8pool", bufs=1)
        xp = xp_ctx.__enter__()
        wv_ctx = tc.tile_pool(name="wvpool", bufs=1)
        wv = wv_ctx.__enter__()
        wqk_ctx = tc.tile_pool(name="wqkpool", bufs=1)
        wqk = wqk_ctx.__enter__()

        st["x8h"] = xp.tile([128, 4, 2, T], F8, tag="x8h", name="x8h")
        st["x8l"] = xp.tile([128, 4, 2, T], F8, tag="x8l", name="x8l")
        st["w8"] = {nm: wqk.tile([128, 4, 2, DQ], F8, tag=nm, name=nm)
                    for nm in ("wqh", "wql", "wkh", "wkl")}
        st["w8v"] = {nm: wv.tile([128, 4, 2, DQ], F8, tag=nm, name=nm)
                     for nm in ("wvh", "wvl")}

        # DMA order: wq first, then x8 c-ascending with wk hoisted after
        # c1 (K_c0 runs ~4 groups in), wv/trim/ident late.  Full-tensor
        # transfers: each dma_start costs ~565ns of SP sequencer, so
        # finer slicing delays later chunks more than it helps the first.
        nc.sync.dma_start(st["w8"]["wqh"][:], w8_d["wqh"])
        nc.gpsimd.dma_start(st["w8"]["wql"][:], w8_d["wql"])
        for c in range(2):
            nc.sync.dma_start(
                st["x8h"][:, :, :, 512 * c:512 * (c + 1)], x8h_d[c])
            nc.gpsimd.dma_start(
                st["x8l"][:, :, :, 512 * c:512 * (c + 1)], x8l_d[c])
        nc.sync.dma_start(st["w8"]["wkh"][:], w8_d["wkh"])
        nc.gpsimd.dma_start(st["w8"]["wkl"][:], w8_d["wkl"])
        for c in range(2, 4):
            nc.sync.dma_start(
                st["x8h"][:, :, :, 512 * c:512 * (c + 1)], x8h_d[c])
            nc.gpsimd.dma_start(
                st["x8l"][:, :, :, 512 * c:512 * (c + 1)], x8l_d[c])
        nc.gpsimd.dma_start(st["trim"][:], tm_d)
        nc.sync.dma_start(st["w8v"]["wvh"][:], w8_d["wvh"])
        nc.gpsimd.dma_start(st["w8v"]["wvl"][:], w8_d["wvl"])
        nc.sync.dma_start(st["ident"][:], id_d)

        late_ctxs = []
        for op in SCHEDULE:
            kind = op[0]
            if kind == "qk":
                emit_qk(op[1])
            elif kind == "qkg":
                emit_qkg(op[1], op[2], op[3])
            elif kind == "s1":
                for r in range(op[2], op[3]):
                    emit_s_chunk(op[1], r, 0)
            elif kind == "s2":
                for r in range(op[2], op[3]):
                    emit_s_chunk(op[1], r, 1)
            elif kind == "s":
                emit_s(op[1], op[2], op[3])
            elif kind == "vp":
                emit_vp(op[1], *op[2:])
            elif kind == "pv":
                emit_pv(op[1], *op[2:])
            elif kind == "tp":
                emit_tp(op[1], *op[2:])
            elif kind == "tail":
                emit_tail(*op[1:])
            elif kind == "close_wqk":
                wqk_ctx.__exit__(None, None, None)
            elif kind == "close_x8":
                wv_ctx.__exit__(None, None, None)
                xp_ctx.__exit__(None, None, None)
            elif kind == "open_ytp":
                # Right-side pool: outlives the left-side x8/wv pools
                # that close later (close_x8), dodging LIFO ordering.
                ytp_ctx = tc.tile_pool(name="ytp", bufs=3, side="right")
                st["ytp"] = ytp_ctx.__enter__()
                late_ctxs.append(ytp_ctx)
            elif kind == "open_late":
                lp_ctx = tc.tile_pool(name="late", bufs=1)
                lp = lp_ctx.__enter__()
                late_ctxs.append(lp_ctx)
                st["lp"] = lp
                st["woT"] = lp.tile([128, NJ, D], F16, tag="woT", name="woT")
                nc.sync.dma_start(
                    st["woT"][:], wo_d.rearrange("(j p) n -> p j n", p=128))
            else:
                raise ValueError(op)
        for ctx in reversed(late_ctxs):
            ctx.__exit__(None, None, None)


def build_nc():
    nc = bacc.Bacc("TRN2", target_bir_lowering=False, debug=False)
    with tile.TileContext(nc) as tc:
        _body(tc)
    nc.compile()
    return nc


_nc_cache = None


def _get_nc():
    global _nc_cache
    if _nc_cache is None:
        _nc_cache = build_nc()
    return _nc_cache


def make_in_maps(x, W_q, W_k, W_v, W_o):
    x = np.asarray(x, dtype=np.float32)
    W_q = np.asarray(W_q, dtype=np.float32)
    W_k = np.asarray(W_k, dtype=np.float32)
    W_v = np.asarray(W_v, dtype=np.float32)
    W_o = np.asarray(W_o, dtype=np.float32)
    in_maps = []
    for c in range(8):
        b, g = divmod(c, 2)
        sl = slice(DQ * g, DQ * (g + 1))
        im = {"trimask": TRIMASK, "ident": IDENT}
        xs = np.ascontiguousarray(x[b].T) * X_SCALE
        xh, xl = _split8(xs)
        for nm, a in (("x8h", xh), ("x8l", xl)):
            im[nm] = np.ascontiguousarray(
                a.reshape(4, 2, 128, 4, 512).transpose(3, 2, 0, 1, 4)
            )
        for nm, W, wsc in (("wq", W_q, W_SCALE), ("wk", W_k, W_SCALE),
                           ("wv", W_v, W_SCALE_V)):
            Ws = np.ascontiguousarray(W[sl].T) * wsc
            wh, wl = _split8(Ws)
            for suf, a in (("h", wh), ("l", wl)):
                im[nm + suf] = np.ascontiguousarray(
                    a.reshape(4, 2, 128, DQ).transpose(2, 0, 1, 3)
                )
        im["wot"] = np.ascontiguousarray(W_o[:, sl].T).astype(np.float16)
        in_maps.append(im)
    return in_maps


def kernel(x, W_q, W_k, W_v, W_o, b_o):
    global LAST
    nc = _get_nc()
    in_maps = make_in_maps(x, W_q, W_k, W_v, W_o)
    res = bass_utils.run_bass_kernel_spmd(
        nc, in_maps, core_ids=list(range(8)), trace=TRACE
    )
    LAST = res
    parts = [np.asarray(res.results[c]["out"], dtype=np.float32) for c in range(8)]
    b_o = np.asarray(b_o, dtype=np.float32)
    out = np.stack([parts[2 * b] + parts[2 * b + 1] for b in range(4)])
    out *= OUT_DESCALE
    out += b_o[None, None, :]
    return out.astype(np.float32)



# revision 38
# speedup vs baseline: 1.0635x; 1.0346x over previous
"""Causal self-attention kernel for 8 Trainium2 NeuronCores.

Problem (hardcoded): x [4, 2048, 1024], torch-style Linear weights
W_q/W_k/W_v/W_o [1024, 1024], b_o [1024]; 16 heads, head_dim 64,
causal softmax attention, out = attn(x) @ W_o.T + b_o.

Sharding: 8 cores = 4 batches x 2 head-groups (8 heads each).
Each core computes a partial output  y_g @ W_o[:, g].T  for its batch;
the host sums the two head-group partials, rescales, and adds b_o.

Per-core pipeline:
  phase A: QKV projections in fp8e4m3 DoubleRow (2 contraction rows per
           partition, 0.5 PE cycles/row) with two-sided error
           compensation: x ~ x_hi + x_lo, W ~ W_hi + W_lo (all fp8,
           host-split, pre-scaled by 8/64 into e4m3 range), accumulate
           x_hi*W_hi + x_lo*W_hi + x_hi*W_lo in PSUM (12 matmuls per
           512-wide tile = 0.75x the fp16 cost).  q/k land as qT/kT
           [dq, T] fp16; v natural [tok, h, 65] fp16 with ones column,
           produced per head-pair so PV(0)/PV(1) unblock early (they
           gate the exp pipeline via ep-buffer recycling).
  phase B: per head: scores S^T[k, q] fp16 (contraction dh=64) ->
           exp on ACT (scale 2^-21 folds softmax scale and the fp8
           pre-scales) into fp16 ep tiles [keys, q-span], causal span
           exact at 128 granularity; diagonal blocks masked via
           triangular mask-multiply on Pool (SBUF only - Pool cannot
           touch PSUM on TRN2).
  phase C: PV with moving dim = dh: y[q, 65] = sum_r ep_r^T @ v_r
           (65 PE cycles per causal block); col 64 is the softmax
           denominator; normalize via DVE reciprocal + DVE
           tensor_scalar (per-partition scalar, extra 0.25 fold);
           head-pair transposes on PE into yT (PSUM->SBUF move by DMA).
  phase D: out accumulated in fp16 SBUF: stage A fuses yT_{0,1,2} @ Wo
           in one PSUM group after pair 2, stage B adds yT_3 @ Wo in a
           per-query-tile tail pipeline woven with PV(7)/transpose,
           DMA out fp16 on both queues (host rescales by 1/128).

Emission order is schedule-driven (SCHEDULE below) so per-engine
program order can be tuned against the cost model.
"""

import numpy as np
import ml_dtypes

import concourse.bass as bass
import concourse.tile as tile
import concourse.mybir as mybir
from concourse import bacc
from concourse import bass_utils

T = 2048
D = 1024
HPC = 8            # heads per core
DH = 64
DQ = HPC * DH      # 512, per-core projection width
NT = T // 128      # 16 row tiles
NJ = DQ // 128     # 4 dq tiles

F32 = mybir.dt.float32
F16 = mybir.dt.float16
F8 = mybir.dt.float8e4
EXP = mybir.ActivationFunctionType.Exp
DR = mybir.MatmulPerfMode.DoubleRow
MUL = mybir.AluOpType.mult
E4NP = ml_dtypes.float8_e4m3

X_SCALE = 8.0      # host pre-scale of x before fp8 split
W_SCALE = 64.0     # host pre-scale of W_{q,k} before fp8 split
W_SCALE_V = 64.0   # W_v pre-scale (fp8 split precision)
EXP_SCALE = 0.125 / (X_SCALE * W_SCALE) ** 2   # == 2^-21, exact
DEN_COL = 4.0      # denominator column value: folds the 0.25 normalize
OUT_DESCALE = DEN_COL / (X_SCALE * W_SCALE_V)  # host undo (1/128)

TRACE = False
LAST = None        # BassKernelResults of the most recent run

TRIMASK = np.triu(np.ones((128, 128), dtype=np.float16))
IDENT = np.eye(128, dtype=np.float16)

# ep tag bufs: big tiles (small r) need depth 3 so exp(h) is only gated
# by PV(h-3); late-emitted small tiles can ride at depth 2 (their gate,
# PV(h-2)-end, passes long before their emission slot).
EP_BUFS = {r: (3 if r < 8 else 2) for r in range(NT)}

# Emission order (per-engine program order).  Ops:
#   ("qk", j)         Q+K projection, dq-tile j (c-descending)
#   ("s", h)          scores + exp + diag mask, head h
#   ("vp", p)         V projection, head pair p (all 16 token tiles)
#   ("pv", h)         attention*V + normalize, head h
#   ("tp", j)         pair transpose into yT_j (j <= 2)
#   ("oja",)          out-proj stage A: yT_{0,1,2} fused PSUM groups
#   ("tail",)         woven PV(7)+transpose+out-proj stage B+DMA
#   ("close_wqk",) ("close_x8",) ("open_late",)   pool scope markers
SCHEDULE = [
    # qk(0) interleaved with s(0)'s first chunks: the exp stream on ACT
    # (the critical engine through ~80% of the kernel) starts ~6us
    # earlier than emitting qk(0) en bloc.
    ("qkg", 0, 0, 0), ("qkg", 0, 1, 0),
    ("s1", 0, 0, 1),
    ("qkg", 0, 0, 1), ("s1", 0, 1, 2),
    ("qkg", 0, 0, 2), ("s1", 0, 2, 4),
    ("qkg", 0, 1, 1), ("s1", 0, 4, 6),
    ("qkg", 0, 0, 3), ("s1", 0, 6, 8),
    ("qkg", 0, 1, 2), ("s2", 0, 0, 4),
    ("qkg", 0, 1, 3), ("s2", 0, 4, 8),
    ("s", 0, 8, NT),
    # s(1) (same qT/kT block as s(0)) interleaved through qk(1) so ACT
    # never drains dry between heads.
    ("qkg", 1, 0, 0), ("qkg", 1, 0, 1),
    ("s1", 1, 0, 2), ("qkg", 1, 1, 0),
    ("s1", 1, 2, 4), ("qkg", 1, 0, 2),
    ("s1", 1, 4, 6), ("qkg", 1, 1, 1),
    ("s1", 1, 6, 8), ("qkg", 1, 0, 3),
    ("s2", 1, 0, 4), ("qkg", 1, 1, 2),
    ("s2", 1, 4, 8), ("qkg", 1, 1, 3),
    ("s", 1, 8, NT),
    ("vp", 0), ("pv", 0),
    ("s1", 2, 0, 2), ("qkg", 2, 0, 0),
    ("s1", 2, 2, 4), ("qkg", 2, 0, 1),
    ("s1", 2, 4, 6), ("qkg", 2, 1, 0),
    ("s1", 2, 6, 8), ("qkg", 2, 0, 2),
    ("s2", 2, 0, 4), ("qkg", 2, 1, 1),
    ("s2", 2, 4, 8), ("qkg", 2, 0, 3),
    ("s", 2, 8, 12), ("qkg", 2, 1, 2),
    ("s", 2, 12, NT), ("qkg", 2, 1, 3),
    # Mid-stream rebalance: each s(h) block carries just enough filler
    # that PE delivers s(h)'s scores exactly when ACT's in-order exp
    # stream needs them (~19.4us per head).  qk(3) is only needed by
    # s(6)/s(7), so it fills s(5); the V-projections and pv's cascade
    # one block earlier than their consumers.
    ("s1", 3, 0, 2), ("vp", 1, 0, 8),
    ("s1", 3, 2, 4), ("vp", 1, 8, 16),
    ("s1", 3, 4, 6), ("pv", 1, 0, 8),
    ("s1", 3, 6, 8), ("pv", 1, 8, 16),
    ("s2", 3, 0, 4), ("vp", 2, 0, 4),
    ("s2", 3, 4, 8), ("vp", 2, 4, 8),
    ("s", 3, 8, 12), ("vp", 2, 8, 12),
    ("s", 3, 12, NT), ("vp", 2, 12, 16),
    ("s1", 4, 0, 2), ("pv", 2, 0, 8),
    ("s1", 4, 2, 4), ("pv", 2, 8, 16),
    ("s1", 4, 4, 6), ("vp", 3, 0, 4),
    ("s1", 4, 6, 8), ("vp", 3, 4, 8),
    ("s2", 4, 0, 4), ("vp", 3, 8, 12),
    ("s2", 4, 4, 8), ("vp", 3, 12, 16),
    ("close_x8",),
    ("s", 4, 8, 12), ("pv", 3, 0, 8),
    ("s", 4, 12, NT), ("pv", 3, 8, 12),
    ("s1", 5, 0, 2), ("pv", 3, 12, 16),
    ("s1", 5, 2, 4), ("qkg", 3, 0, 0),
    ("s1", 5, 4, 6), ("qkg", 3, 0, 1),
    ("s1", 5, 6, 8), ("qkg", 3, 1, 0),
    ("s2", 5, 0, 4), ("qkg", 3, 0, 2),
    ("s2", 5, 4, 8), ("qkg", 3, 1, 1),
    ("s", 5, 8, 12), ("qkg", 3, 0, 3), ("qkg", 3, 1, 2),
    ("s", 5, 12, NT), ("qkg", 3, 1, 3),
    ("close_wqk",), ("open_ytp",),
    ("s1", 6, 0, 2), ("tp", 0, 0, 8),
    ("s1", 6, 2, 4), ("tp", 0, 8, 16),
    ("s1", 6, 4, 6), ("pv", 4, 0, 4),
    ("s1", 6, 6, 8), ("pv", 4, 4, 8),
    ("open_late",),
    ("s2", 6, 0, 4), ("pv", 4, 8, 16),
    ("s2", 6, 4, 8), ("tp", 1, 0, 8),
    ("s", 6, 8, 12), ("tp", 1, 8, 16), ("pv", 5, 0, 4),
    ("s", 6, 12, NT), ("pv", 5, 4, 8),
    ("s1", 7, 0, 2), ("pv", 5, 8, 16),
    ("s1", 7, 2, 4), ("tp", 2, 0, 8),
    ("s1", 7, 4, 6), ("tp", 2, 8, 16),
    ("s1", 7, 6, 8), ("pv", 6, 0, 8),
    ("tail", 0, 2),
    ("s2", 7, 0, 4), ("pv", 6, 8, 16),
    ("tail", 2, 4),
    ("s2", 7, 4, 8), ("tail", 4, 6),
    ("s", 7, 8, 10), ("tail", 6, 8),
    ("s", 7, 10, 12), ("tail", 8, 10),
    ("s", 7, 12, 14), ("tail", 10, 12),
    ("s", 7, 14, NT), ("tail", 12, NT),
]


def _split8(a):
    """Two-sided fp8 split: a ~ hi + lo, both e4m3."""
    hi = np.clip(a, -240, 240).astype(E4NP)
    lo = np.clip(a - hi.astype(np.float32), -240, 240).astype(E4NP)
    return hi, lo


def _body(tc):
    nc = tc.nc
    # fp8 x: [c, p, s, i, col]: row 256s+128i+p of x.T*8, col 512c+col
    x8h_d = nc.dram_tensor("x8h", (4, 128, 4, 2, 512), F8, kind="ExternalInput").ap()
    x8l_d = nc.dram_tensor("x8l", (4, 128, 4, 2, 512), F8, kind="ExternalInput").ap()
    w8_d = {}
    for nm in ("wqh", "wql", "wkh", "wkl", "wvh", "wvl"):
        w8_d[nm] = nc.dram_tensor(nm, (128, 4, 2, DQ), F8, kind="ExternalInput").ap()
    wo_d = nc.dram_tensor("wot", (DQ, D), F16, kind="ExternalInput").ap()
    tm_d = nc.dram_tensor("trimask", (128, 128), F16, kind="ExternalInput").ap()
    id_d = nc.dram_tensor("ident", (128, 128), F16, kind="ExternalInput").ap()
    out_d = nc.dram_tensor("out", (T, D), F16, kind="ExternalOutput").ap()

    st = {}  # emission state

    def emit_qkg(j, qk, c):
        # One projection psum group: dq-block j, Q (qk=0) or K (qk=1),
        # token chunk c.
        x8h, x8l, w8 = st["x8h"], st["x8l"], st["w8"]
        if qk == 0:
            dest, wh, wl = st["qT"], w8["wqh"], w8["wql"]
        else:
            dest, wh, wl = st["kT"], w8["wkh"], w8["wkl"]
        terms = ((x8h, wh), (x8l, wh), (x8h, wl))
        ps = st["flex"].tile([128, 512], F32, tag="fx", name=f"p{j}_{c}")
        ki = 0
        for (xs, ws) in terms:
            for s in range(4):
                nc.tensor.matmul(
                    ps[:],
                    ws[:, s, :, 128 * j:128 * (j + 1)],
                    xs[:, s, :, 512 * c:512 * (c + 1)],
                    start=(ki == 0), stop=(ki == 11),
                    perf_mode=DR,
                )
                ki += 1
        nc.vector.tensor_copy(dest[:, j, 512 * c:512 * (c + 1)], ps[:])

    def emit_qk(j):
        # c-ascending with K's early blocks hoisted: scores consume
        # qT (all c) and kT key-block r, so Q chunks and low-c K chunks
        # unblock score tiles soonest.
        for qk, c in ((0, 0), (0, 1), (1, 0), (0, 2),
                      (1, 1), (0, 3), (1, 2), (1, 3)):
            emit_qkg(j, qk, c)

    def emit_vp(p, t0=0, t1=NT):
        # V projection for head pair p: out columns [128p, 128p+128)
        x8h, x8l, w8v = st["x8h"], st["x8l"], st["w8v"]
        terms = ((x8h, w8v["wvh"]), (x8l, w8v["wvh"]), (x8h, w8v["wvl"]))
        for t in range(t0, t1):
            ps = st["flex"].tile([128, 128], F32, tag="fx", name=f"pv{p}_{t}")
            ki = 0
            for (xs, ws) in terms:
                for s in range(4):
                    nc.tensor.matmul(
                        ps[:],
                        xs[:, s, :, 128 * t:128 * (t + 1)],
                        ws[:, s, :, 128 * p:128 * (p + 1)],
                        start=(ki == 0), stop=(ki == 11),
                        perf_mode=DR,
                    )
                    ki += 1
            nc.vector.tensor_copy(
                st["v"][:, t, 2 * p:2 * p + 2, 0:DH],
                ps[:].rearrange("p (h d) -> p h d", h=2),
            )

    def emit_s_chunk(h, r, chunk):
        # chunk 0: q-span (s0, 1536) [or (s0, 2048) for r >= 4] + diag
        # mask; chunk 1: q-span (1536, 2048) for r < 4.  1536-wide stt
        # slots (3 PSUM banks x 2 bufs) give 160 activations instead of
        # 192 (each ACT instruction pays ~143ns of PSUM access) at the
        # same PE-ahead-of-ACT lookahead of 3072 score columns.
        qT, kT, trim = st["qT"], st["kT"], st["trim"]
        hp = (h % 2) * DH
        hj = h // 2
        s0 = 128 * r
        if chunk == 0:
            ep = st["epool"].tile([128, T - s0], F16, tag=f"e{r}",
                                  name=f"e{r}_h{h}", bufs=EP_BUFS[r])
            st["ep"][(h, r)] = ep
            lo, hi = s0, (1024 if r < 8 else 2048)
        else:
            ep = st["ep"][(h, r)]
            lo, hi = 1024, 2048
        stt = st["stp"].tile([128, hi - lo], F32, tag="st",
                             padded_shape=[128, 1024])
        pieces = ([(lo, 512), (512, hi)] if (h, r, chunk) == (0, 0, 0)
                  else [(lo, hi)])
        for (alo, ahi) in pieces:
            for n0 in range(alo, ahi, 512):
                n1 = min(n0 + 512, ahi)
                nc.tensor.matmul(
                    stt[:, n0 - lo:n1 - lo],
                    kT[hp:hp + DH, hj, s0:s0 + 128],
                    qT[hp:hp + DH, hj, n0:n1],
                    start=True, stop=True,
                )
            nc.scalar.activation(
                ep[:, alo - s0:ahi - s0], stt[:, alo - lo:ahi - lo],
                EXP, scale=EXP_SCALE,
            )
        if chunk == 0:
            nc.gpsimd.tensor_mul(ep[:, 0:128], ep[:, 0:128], trim[:])

    def emit_s(h, r0=0, r1=NT):
        for r in range(r0, r1):
            emit_s_chunk(h, r, 0)
            if r < 8:
                emit_s_chunk(h, r, 1)

    def pv_quad(h, q0, nq):
        """PV psum groups for nq query tiles sharing one PSUM tile, with
        a single strided reciprocal + broadcast-multiply normalize.  The
        0.25 fold lives in the host-side W_v scale, so the normalize is
        a pure multiply and the per-tile DVE chain (two ops per query
        tile) collapses to two ops per quad."""
        v, yn = st["v"], st["yn"]
        par = (h // 2) % 2
        hq = h % 2
        ya = st["flex"].tile([128, nq, DH + 1], F32, tag="fx",
                             name=f"y{h}_{q0}", padded_shape=[128, 4, DH + 1])
        for qi in range(nq):
            qt = q0 + qi
            for r in range(qt + 1):
                ep = st["ep"][(h, r)]
                off = 128 * (qt - r)
                nc.tensor.matmul(
                    ya[:, qi, :], ep[:, off:off + 128], v[:, r, h, :],
                    start=(r == 0), stop=(r == qt),
                )
        rec = st["sp"].tile([128, nq], F32, tag="rec", name=f"rec{h}_{q0}",
                            padded_shape=[128, 4])
        nc.vector.reciprocal(rec[:], ya[:, :, DH])
        rb = rec[:].unsqueeze(2).broadcast_to((128, nq, DH))
        nc.vector.tensor_tensor(
            yn[:, par, q0:q0 + nq, hq, :], ya[:, :, 0:DH], rb, MUL,
        )

    def emit_pv(h, qt0=0, qt1=NT):
        for q0 in range(qt0, qt1, 4):
            pv_quad(h, q0, min(4, qt1 - q0))
        if qt1 == NT:
            for r in range(NT):
                del st["ep"][(h, r)]

    def tp_tile(j, qt, yTj):
        tp = st["flex"].tile([128, 128], F16, tag="fx", name=f"tp{j}_{qt}")
        nc.tensor.transpose(tp[:], st["yn"][:, j % 2, qt, :, :], st["ident"][:])
        nc.vector.tensor_copy(yTj[:, 128 * qt:128 * (qt + 1)], tp[:])

    def emit_tp(j, q0=0, q1=NT):
        if q0 == 0:
            st["yT"][j] = st["ytp"].tile([128, T], F16, tag="yT",
                                         name=f"yT{j}")
        yTj = st["yT"][j]
        for qt in range(q0, q1):
            tp_tile(j, qt, yTj)

    def emit_tail(qt0=0, qt1=NT):
        # Software-pipelined tail, lag 2: stage 1 at qt = PV(7, qt) +
        # pair-3 transpose into yT3[qt]; stage 2 at qt-2 = the WHOLE
        # out projection (one 8-matmul psum group per qt) -> psum->sbuf
        # copies on ACT/DVE -> DMA out.  Range form so the schedule can
        # weave tail stages between s(7) chunks: oj(qt) starts as soon
        # as exp(7, qt) lands instead of after the whole s(7) block.
        woT = st["woT"]
        yT3s = st.setdefault("yT3s", {})

        def oj_qt(q):
            # Both 512-halves accumulate in one 2-bank stp tile (stp is
            # score-free by the tail); the PSUM->SBUF copies split across
            # ACT (idle after the last exp) and DVE so neither serializes
            # the per-qt chain.
            ps = st["stp"].tile([128, 1024], F32, tag="st", name=f"ob{q}")
            for d in range(2):
                for j in range(4):
                    lhs = (st["yT"][j][:, 128 * q:128 * (q + 1)]
                           if j < 3 else yT3s[q][:])
                    nc.tensor.matmul(
                        ps[:, 512 * d:512 * (d + 1)], lhs,
                        woT[:, j, 512 * d:512 * (d + 1)],
                        start=(j == 0), stop=(j == 3),
                    )
                # d=0 copy on ACT (mostly idle by the tail) overlaps the
                # d=1 matmuls; d=1 copy on DVE so neither serializes.
                ob = st["lp"].tile([128, 512], F16, tag="ob",
                                   name=f"ob{q}_{d}", bufs=4)
                if d == 0:
                    nc.scalar.copy(ob[:], ps[:, 0:512])
                else:
                    nc.vector.tensor_copy(ob[:], ps[:, 512:1024])
                qu = nc.sync if d == 0 else nc.gpsimd
                orow = out_d[128 * q:128 * (q + 1), 512 * d:512 * (d + 1)]
                if q == NT - 1:
                    # Final group: halve the last transfers so the
                    # end-of-kernel DMA+sem chain is shorter.
                    qu.dma_start(orow[:, 0:256], ob[:, 0:256])
                    qu2 = nc.gpsimd if d == 0 else nc.sync
                    qu2.dma_start(orow[:, 256:512], ob[:, 256:512])
                else:
                    qu.dma_start(orow, ob[:])
            del yT3s[q]

        for qt in range(qt0, qt1):
            pv_quad(7, qt, 1)
            yT3s[qt] = st["lp"].tile([128, 128], F16, tag="yt3",
                                     name=f"yt3_{qt}", bufs=4)
            tp = st["flex"].tile([128, 128], F16, tag="fx", name=f"tp3_{qt}")
            nc.tensor.transpose(tp[:], st["yn"][:, 1, qt, :, :], st["ident"][:])
            nc.vector.tensor_copy(yT3s[qt][:], tp[:])
            if qt >= 2:
                oj_qt(qt - 2)
        if qt1 == NT:
            oj_qt(NT - 2)
            oj_qt(NT - 1)
            for r in range(NT):
                del st["ep"][(7, r)]

    with (
        tc.tile_pool(name="persist", bufs=1) as pp,
        tc.tile_pool(name="expp", bufs=3) as epool,
        tc.tile_pool(name="psum_st", bufs=3, space="PSUM") as stp,
        tc.tile_pool(name="psum_fx", bufs=2, space="PSUM") as flex,
        tc.tile_pool(name="small", bufs=4) as sp,
    ):
        st["qT"] = pp.tile([128, NJ, T], F16, tag="qT", name="qT")
        st["kT"] = pp.tile([128, NJ, T], F16, tag="kT", name="kT")
        st["v"] = pp.tile([128, NT, HPC, DH + 1], F16, tag="v", name="v")
        st["trim"] = pp.tile([128, 128], F16, tag="trim", name="trim")
        st["ident"] = pp.tile([128, 128], F16, tag="ident", name="ident")
        # normalized y staging: [pair parity, qt, head parity, dh]
        st["yn"] = pp.tile([128, 2, NT, 2, DH], F16, tag="yn", name="yn")
        st["epool"], st["stp"], st["flex"], st["sp"] = epool, stp, flex, sp
        st["ep"], st["yT"] = {}, {}

        warm = pp.tile([1, DH], F16, tag="warm", name="warm")
        nc.gpsimd.memset(warm[:], 1.0)
        nc.gpsimd.memset(st["v"][:, :, :, DH:DH + 1], DEN_COL)
        nc.scalar.activation(warm[:], warm[:], EXP, scale=1.0)

        # phase-A pools, closed by schedule markers (LIFO: wqk on top)
        wqk_ctx = tc.tile_pool(name="wqkpool", bufs=1)
        wqk = wqk_ctx.__enter__()
        xp_ctx = tc.tile_pool(name="x8pool", bufs=1)
        xp = xp_ctx.__enter__()
        wv_ctx = tc.tile_pool(name="wvpool", bufs=1)
        wv = wv_ctx.__enter__()

        st["x8h"] = xp.tile([128, 4, 2, T], F8, tag="x8h", name="x8h")
        st["x8l"] = xp.tile([128, 4, 2, T], F8, tag="x8l", name="x8l")
        st["w8"] = {nm: wqk.tile([128, 4, 2, DQ], F8, tag=nm, name=nm)
                    for nm in ("wqh", "wql", "wkh", "wkl")}
        st["w8v"] = {nm: wv.tile([128, 4, 2, DQ], F8, tag=nm, name=nm)
                     for nm in ("wvh", "wvl")}

        # DMA order: wq first, then x8 c-ascending with wk hoisted after
        # c1 (K_c0 runs ~4 groups in), wv/trim/ident late.  Full-tensor
        # transfers: each dma_start costs ~565ns of SP sequencer, so
        # finer slicing delays later chunks more than it helps the first.
        nc.sync.dma_start(st["w8"]["wqh"][:], w8_d["wqh"])
        nc.gpsimd.dma_start(st["w8"]["wql"][:], w8_d["wql"])
        for c in range(1):
            nc.sync.dma_start(
                st["x8h"][:, :, :, 512 * c:512 * (c + 1)], x8h_d[c])
            nc.gpsimd.dma_start(
                st["x8l"][:, :, :, 512 * c:512 * (c + 1)], x8l_d[c])
        nc.sync.dma_start(st["w8"]["wkh"][:], w8_d["wkh"])
        nc.gpsimd.dma_start(st["w8"]["wkl"][:], w8_d["wkl"])
        for c in range(1, 4):
            nc.sync.dma_start(
                st["x8h"][:, :, :, 512 * c:512 * (c + 1)], x8h_d[c])
            nc.gpsimd.dma_start(
                st["x8l"][:, :, :, 512 * c:512 * (c + 1)], x8l_d[c])
        nc.gpsimd.dma_start(st["trim"][:], tm_d)
        nc.sync.dma_start(st["w8v"]["wvh"][:], w8_d["wvh"])
        nc.gpsimd.dma_start(st["w8v"]["wvl"][:], w8_d["wvl"])
        nc.sync.dma_start(st["ident"][:], id_d)

        late_ctxs = []
        for op in SCHEDULE:
            kind = op[0]
            if kind == "qk":
                emit_qk(op[1])
            elif kind == "qkg":
                emit_qkg(op[1], op[2], op[3])
            elif kind == "s1":
                for r in range(op[2], op[3]):
                    emit_s_chunk(op[1], r, 0)
            elif kind == "s2":
                for r in range(op[2], op[3]):
                    emit_s_chunk(op[1], r, 1)
            elif kind == "s":
                emit_s(op[1], op[2], op[3])
            elif kind == "vp":
                emit_vp(op[1], *op[2:])
            elif kind == "pv":
                emit_pv(op[1], *op[2:])
            elif kind == "tp":
                emit_tp(op[1], *op[2:])
            elif kind == "tail":
                emit_tail(*op[1:])
            elif kind == "close_wqk":
                wqk_ctx.__exit__(None, None, None)
            elif kind == "close_x8":
                wv_ctx.__exit__(None, None, None)
                xp_ctx.__exit__(None, None, None)
            elif kind == "open_ytp":
                # Right-side pool: outlives the left-side x8/wv pools
                # that close later (close_x8), dodging LIFO ordering.
                ytp_ctx = tc.tile_pool(name="ytp", bufs=3, side="right")
                st["ytp"] = ytp_ctx.__enter__()
                late_ctxs.append(ytp_ctx)
            elif kind == "open_late":
                lp_ctx = tc.tile_pool(name="late", bufs=1)
                lp = lp_ctx.__enter__()
                late_ctxs.append(lp_ctx)
                st["lp"] = lp
                st["woT"] = lp.tile([128, NJ, D], F16, tag="woT", name="woT")
                nc.sync.dma_start(
                    st["woT"][:], wo_d.rearrange("(j p) n -> p j n", p=128))
            else:
                raise ValueError(op)
        for ctx in reversed(late_ctxs):
            ctx.__exit__(None, None, None)


def build_nc():
    nc = bacc.Bacc("TRN2", target_bir_lowering=False, debug=False)
    with tile.TileContext(nc) as tc:
        _body(tc)
    nc.compile()
    return nc


_nc_cache = None


def _get_nc():
    global _nc_cache
    if _nc_cache is None:
        _nc_cache = build_nc()
    return _nc_cache


def make_in_maps(x, W_q, W_k, W_v, W_o):
    x = np.asarray(x, dtype=np.float32)
    W_q = np.asarray(W_q, dtype=np.float32)
    W_k = np.asarray(W_k, dtype=np.float32)
    W_v = np.asarray(W_v, dtype=np.float32)
    W_o = np.asarray(W_o, dtype=np.float32)
    in_maps = []
    for c in range(8):
        b, g = divmod(c, 2)
        sl = slice(DQ * g, DQ * (g + 1))
        im = {"trimask": TRIMASK, "ident": IDENT}
        xs = np.ascontiguousarray(x[b].T) * X_SCALE
        xh, xl = _split8(xs)
        for nm, a in (("x8h", xh), ("x8l", xl)):
            im[nm] = np.ascontiguousarray(
                a.reshape(4, 2, 128, 4, 512).transpose(3, 2, 0, 1, 4)
            )
        for nm, W, wsc in (("wq", W_q, W_SCALE), ("wk", W_k, W_SCALE),
                           ("wv", W_v, W_SCALE_V)):
            Ws = np.ascontiguousarray(W[sl].T) * wsc
            wh, wl = _split8(Ws)
            for suf, a in (("h", wh), ("l", wl)):
                im[nm + suf] = np.ascontiguousarray(
                    a.reshape(4, 2, 128, DQ).transpose(2, 0, 1, 3)
                )
        im["wot"] = np.ascontiguousarray(W_o[:, sl].T).astype(np.float16)
        in_maps.append(im)
    return in_maps


def kernel(x, W_q, W_k, W_v, W_o, b_o):
    global LAST
    nc = _get_nc()
    in_maps = make_in_maps(x, W_q, W_k, W_v, W_o)
    res = bass_utils.run_bass_kernel_spmd(
        nc, in_maps, core_ids=list(range(8)), trace=TRACE
    )
    LAST = res
    parts = [np.asarray(res.results[c]["out"], dtype=np.float32) for c in range(8)]
    b_o = np.asarray(b_o, dtype=np.float32)
    out = np.stack([parts[2 * b] + parts[2 * b + 1] for b in range(4)])
    out *= OUT_DESCALE
    out += b_o[None, None, :]
    return out.astype(np.float32)



# revision 48
# speedup vs baseline: 1.0714x; 1.0074x over previous
"""Causal self-attention kernel for 8 Trainium2 NeuronCores.

Problem (hardcoded): x [4, 2048, 1024], torch-style Linear weights
W_q/W_k/W_v/W_o [1024, 1024], b_o [1024]; 16 heads, head_dim 64,
causal softmax attention, out = attn(x) @ W_o.T + b_o.

Sharding: 8 cores = 4 batches x 2 head-groups (8 heads each).
Each core computes a partial output  y_g @ W_o[:, g].T  for its batch;
the host sums the two head-group partials, rescales, and adds b_o.

Per-core pipeline:
  phase A: QKV projections in fp8e4m3 DoubleRow (2 contraction rows per
           partition, 0.5 PE cycles/row) with two-sided error
           compensation: x ~ x_hi + x_lo, W ~ W_hi + W_lo (all fp8,
           host-split, pre-scaled by 8/64 into e4m3 range), accumulate
           x_hi*W_hi + x_lo*W_hi + x_hi*W_lo in PSUM (12 matmuls per
           512-wide tile = 0.75x the fp16 cost).  q/k land as qT/kT
           [dq, T] fp16; v natural [tok, h, 65] fp16 with a constant
           DEN_COL column that folds the 0.25 normalize factor.
  phase B: per head: scores S^T[k, q] fp16 (contraction dh=64) ->
           exp on ACT (scale 2^-21 folds softmax scale and the fp8
           pre-scales) into fp16 ep tiles [keys, q-span], causal span
           exact at 128 granularity; diagonal blocks masked via
           triangular mask-multiply on Pool (SBUF only - Pool cannot
           touch PSUM on TRN2).
  phase C: PV with moving dim = dh: y[q, 65] = sum_r ep_r^T @ v_r
           (65 PE cycles per causal block); col 64 is the softmax
           denominator; four query tiles share one PSUM tile so the
           normalize is a single strided DVE reciprocal + one
           broadcast tensor_tensor multiply per quad; head-pair
           transposes on PE into yT fp16.
  phase D: out projection per query tile: one 2-bank PSUM group (both
           512-halves), PSUM->SBUF copies split ACT/DVE, DMA out fp16
           on both queues (host rescales by 1/128), software-pipelined
           at lag 2 behind PV(7)/transpose and woven into s(7).

ACT (the exp stream, ~153us busy) and PE (~181.5us busy) are the two
near-critical engines.  ACT processes exps in order at ~19.4us/head
and both couple through the 3-deep score-PSUM pool and the ep-tile
ring, so SCHEDULE paces every head's score chunks to land exactly at
ACT's demand time and weaves all other PE work (projections, PV,
transposes, out-proj) between them; with that pacing PE runs hole-free
from first DMA landing to the final out-projection.
"""

import numpy as np
import ml_dtypes

import concourse.bass as bass
import concourse.tile as tile
import concourse.mybir as mybir
from concourse import bacc
from concourse import bass_utils

T = 2048
D = 1024
HPC = 8            # heads per core
DH = 64
DQ = HPC * DH      # 512, per-core projection width
NT = T // 128      # 16 row tiles
NJ = DQ // 128     # 4 dq tiles

F32 = mybir.dt.float32
F16 = mybir.dt.float16
F8 = mybir.dt.float8e4
EXP = mybir.ActivationFunctionType.Exp
DR = mybir.MatmulPerfMode.DoubleRow
MUL = mybir.AluOpType.mult
E4NP = ml_dtypes.float8_e4m3

X_SCALE = 8.0      # host pre-scale of x before fp8 split
W_SCALE = 64.0     # host pre-scale of W_{q,k} before fp8 split
W_SCALE_V = 64.0   # W_v pre-scale (fp8 split precision)
EXP_SCALE = 0.125 / (X_SCALE * W_SCALE) ** 2   # == 2^-21, exact
DEN_COL = 4.0      # denominator column value: folds the 0.25 normalize
OUT_DESCALE = DEN_COL / (X_SCALE * W_SCALE_V)  # host undo (1/128)
SUB = mybir.AluOpType.subtract

TRACE = False
LAST = None        # BassKernelResults of the most recent run

TRIMASK = np.triu(np.ones((128, 128), dtype=np.float16))
IDENT = np.eye(128, dtype=np.float16)

# ep tag bufs: big tiles (small r) need depth 3 so exp(h) is only gated
# by PV(h-3); late-emitted small tiles can ride at depth 2 (their gate,
# PV(h-2)-end, passes long before their emission slot).
EP_BUFS = {r: (3 if r < 8 else 2) for r in range(NT)}

# Emission order (per-engine program order).  Ops:
#   ("qk", j)         Q+K projection, dq-tile j (c-descending)
#   ("s", h)          scores + exp + diag mask, head h
#   ("vp", p)         V projection, head pair p (all 16 token tiles)
#   ("pv", h)         attention*V + normalize, head h
#   ("tp", j)         pair transpose into yT_j (j <= 2)
#   ("oja",)          out-proj stage A: yT_{0,1,2} fused PSUM groups
#   ("tail",)         woven PV(7)+transpose+out-proj stage B+DMA
#   ("close_wqk",) ("close_x8",) ("open_late",)   pool scope markers
SCHEDULE = [
    # qk(0) interleaved with s(0)'s first chunks: the exp stream on ACT
    # (the critical engine through ~80% of the kernel) starts ~6us
    # earlier than emitting qk(0) en bloc.
    ("qkg", 0, 0, 0), ("qkg", 0, 1, 0),
    ("s1", 0, 0, 1),
    ("qkg", 0, 0, 1), ("s1", 0, 1, 2),
    ("qkg", 0, 0, 2), ("s1", 0, 2, 4),
    ("qkg", 0, 1, 1), ("s1", 0, 4, 6),
    ("qkg", 0, 0, 3), ("s1", 0, 6, 8),
    ("qkg", 0, 1, 2), ("s2", 0, 0, 4),
    ("qkg", 0, 1, 3), ("s2", 0, 4, 8),
    ("s", 0, 8, NT),
    # s(1) (same qT/kT block as s(0)) interleaved through qk(1) so ACT
    # never drains dry between heads.
    ("qkg", 1, 0, 0), ("qkg", 1, 0, 1),
    ("s1", 1, 0, 2), ("qkg", 1, 1, 0),
    ("s1", 1, 2, 4), ("qkg", 1, 0, 2),
    ("s1", 1, 4, 6), ("qkg", 1, 1, 1),
    ("s1", 1, 6, 8), ("qkg", 1, 0, 3),
    ("s2", 1, 0, 4), ("qkg", 1, 1, 2),
    ("s2", 1, 4, 8), ("qkg", 1, 1, 3),
    ("s", 1, 8, NT),
    # Paced mid-stream: each s(h) block carries just enough filler that
    # PE delivers s(h)'s scores exactly when ACT's in-order exp stream
    # needs them (~19.4us per head).  vp0/pv0 weave into s(2), qk(2)
    # splits across s(2)/s(3), vp3 and qk(3) (needed only by s(6)/s(7))
    # ride in s(5), cascading every producer one block ahead of its
    # consumer.
    ("s1", 2, 0, 2), ("vp", 0, 0, 8),
    ("s1", 2, 2, 4), ("vp", 0, 8, 16),
    ("s1", 2, 4, 6), ("pv", 0, 0, 8),
    ("s1", 2, 6, 8), ("pv", 0, 8, 16),
    ("s2", 2, 0, 4), ("qkg", 2, 0, 0),
    ("s2", 2, 4, 8), ("qkg", 2, 0, 1),
    ("s", 2, 8, 12), ("qkg", 2, 1, 0),
    ("s", 2, 12, NT), ("qkg", 2, 0, 2),
    ("s1", 3, 0, 2), ("pv", 1, 0, 8),
    ("s1", 3, 2, 4), ("pv", 1, 8, 16),
    ("s1", 3, 4, 6), ("qkg", 2, 1, 1),
    ("s1", 3, 6, 8), ("qkg", 2, 0, 3),
    ("s2", 3, 0, 4), ("qkg", 2, 1, 2),
    ("s2", 3, 4, 8), ("qkg", 2, 1, 3),
    ("s", 3, 8, 12), ("vp", 1, 0, 8),
    ("s", 3, 12, NT), ("vp", 1, 8, 16),
    ("s1", 4, 0, 2), ("pv", 2, 0, 8),
    ("s1", 4, 2, 4), ("pv", 2, 8, 16),
    ("s1", 4, 4, 6), ("vp", 2, 0, 4),
    ("s1", 4, 6, 8), ("vp", 2, 4, 8),
    ("s2", 4, 0, 4), ("vp", 2, 8, 12),
    ("s2", 4, 4, 8), ("vp", 2, 12, 16),
    ("s", 4, 8, 12), ("pv", 3, 0, 8),
    ("s", 4, 12, NT), ("pv", 3, 8, 12),
    ("s1", 5, 0, 2), ("pv", 3, 12, 16),
    ("s1", 5, 2, 4), ("vp", 3, 0, 8),
    ("s1", 5, 4, 6), ("vp", 3, 8, 16),
    ("close_x8",),
    ("s1", 5, 6, 8), ("qkg", 3, 0, 0),
    ("s2", 5, 0, 4), ("qkg", 3, 0, 1), ("qkg", 3, 1, 0),
    ("s2", 5, 4, 8), ("qkg", 3, 0, 2),
    ("s", 5, 8, 12), ("qkg", 3, 1, 1),
    ("s", 5, 12, NT), ("qkg", 3, 0, 3),
    ("qkg", 3, 1, 2), ("qkg", 3, 1, 3),
    ("close_wqk",), ("open_ytp",),
    ("s1", 6, 0, 2), ("tp", 0, 0, 8),
    ("s1", 6, 2, 4), ("tp", 0, 8, 16),
    ("s1", 6, 4, 6), ("pv", 4, 0, 4),
    ("s1", 6, 6, 8), ("pv", 4, 4, 8),
    ("open_late",),
    ("s2", 6, 0, 4), ("pv", 4, 8, 16),
    ("s2", 6, 4, 8), ("tp", 1, 0, 8),
    ("s", 6, 8, 12), ("tp", 1, 8, 16), ("pv", 5, 0, 4),
    ("s", 6, 12, NT), ("pv", 5, 4, 8),
    ("s1", 7, 0, 2), ("pv", 5, 8, 16),
    ("s1", 7, 2, 4), ("tp", 2, 0, 8),
    ("s1", 7, 4, 6), ("tp", 2, 8, 16),
    ("s1", 7, 6, 8), ("pv", 6, 0, 8),
    ("tail", 0, 2),
    ("s2", 7, 0, 4), ("pv", 6, 8, 16),
    ("tail", 2, 4),
    ("s2", 7, 4, 8), ("tail", 4, 6),
    ("s", 7, 8, 10), ("tail", 6, 8),
    ("s", 7, 10, 12), ("tail", 8, 10),
    ("s", 7, 12, 14), ("tail", 10, 12),
    ("s", 7, 14, NT), ("tail", 12, NT),
]


def _split8(a):
    """Two-sided fp8 split: a ~ hi + lo, both e4m3."""
    hi = np.clip(a, -240, 240).astype(E4NP)
    lo = np.clip(a - hi.astype(np.float32), -240, 240).astype(E4NP)
    return hi, lo


def _body(tc):
    nc = tc.nc
    # fp8 x: [c, p, s, i, col]: row 256s+128i+p of x.T*8, col 512c+col
    x8h_d = nc.dram_tensor("x8h", (4, 128, 4, 2, 512), F8, kind="ExternalInput").ap()
    x8l_d = nc.dram_tensor("x8l", (4, 128, 4, 2, 512), F8, kind="ExternalInput").ap()
    w8_d = {}
    for nm in ("wqh", "wql", "wkh", "wkl", "wvh", "wvl"):
        w8_d[nm] = nc.dram_tensor(nm, (128, 4, 2, DQ), F8, kind="ExternalInput").ap()
    wo_d = nc.dram_tensor("wot", (DQ, D), F16, kind="ExternalInput").ap()
    tm_d = nc.dram_tensor("trimask", (128, 128), F16, kind="ExternalInput").ap()
    id_d = nc.dram_tensor("ident", (128, 128), F16, kind="ExternalInput").ap()
    out_d = nc.dram_tensor("out", (T, D), F16, kind="ExternalOutput").ap()

    st = {}  # emission state

    def emit_qkg(j, qk, c):
        # One projection psum group: dq-block j, Q (qk=0) or K (qk=1),
        # token chunk c.
        x8h, x8l, w8 = st["x8h"], st["x8l"], st["w8"]
        if qk == 0:
            dest, wh, wl = st["qT"], w8["wqh"], w8["wql"]
        else:
            dest, wh, wl = st["kT"], w8["wkh"], w8["wkl"]
        terms = ((x8h, wh), (x8l, wh), (x8h, wl))
        ps = st["flex"].tile([128, 512], F32, tag="fx", name=f"p{j}_{c}")
        ki = 0
        for (xs, ws) in terms:
            for s in range(4):
                nc.tensor.matmul(
                    ps[:],
                    ws[:, s, :, 128 * j:128 * (j + 1)],
                    xs[:, s, :, 512 * c:512 * (c + 1)],
                    start=(ki == 0), stop=(ki == 11),
                    perf_mode=DR,
                )
                ki += 1
        nc.vector.tensor_copy(dest[:, j, 512 * c:512 * (c + 1)], ps[:])

    def emit_qk(j):
        # c-ascending with K's early blocks hoisted: scores consume
        # qT (all c) and kT key-block r, so Q chunks and low-c K chunks
        # unblock score tiles soonest.
        for qk, c in ((0, 0), (0, 1), (1, 0), (0, 2),
                      (1, 1), (0, 3), (1, 2), (1, 3)):
            emit_qkg(j, qk, c)

    def emit_vp(p, t0=0, t1=NT):
        # V projection for head pair p: out columns [128p, 128p+128)
        x8h, x8l, w8v = st["x8h"], st["x8l"], st["w8v"]
        terms = ((x8h, w8v["wvh"]), (x8l, w8v["wvh"]), (x8h, w8v["wvl"]))
        for t in range(t0, t1):
            ps = st["flex"].tile([128, 128], F32, tag="fx", name=f"pv{p}_{t}")
            ki = 0
            for (xs, ws) in terms:
                for s in range(4):
                    nc.tensor.matmul(
                        ps[:],
                        xs[:, s, :, 128 * t:128 * (t + 1)],
                        ws[:, s, :, 128 * p:128 * (p + 1)],
                        start=(ki == 0), stop=(ki == 11),
                        perf_mode=DR,
                    )
                    ki += 1
            nc.vector.tensor_copy(
                st["v"][:, t, 2 * p:2 * p + 2, 0:DH],
                ps[:].rearrange("p (h d) -> p h d", h=2),
            )

    def emit_s_chunk(h, r, chunk):
        # chunk 0: q-span (s0, 1536) [or (s0, 2048) for r >= 4] + diag
        # mask; chunk 1: q-span (1536, 2048) for r < 4.  1536-wide stt
        # slots (3 PSUM banks x 2 bufs) give 160 activations instead of
        # 192 (each ACT instruction pays ~143ns of PSUM access) at the
        # same PE-ahead-of-ACT lookahead of 3072 score columns.
        qT, kT, trim = st["qT"], st["kT"], st["trim"]
        hp = (h % 2) * DH
        hj = h // 2
        s0 = 128 * r
        if chunk == 0:
            ep = st["epool"].tile([128, T - s0], F16, tag=f"e{r}",
                                  name=f"e{r}_h{h}", bufs=EP_BUFS[r])
            st["ep"][(h, r)] = ep
            lo, hi = s0, (1024 if r < 8 else 2048)
        else:
            ep = st["ep"][(h, r)]
            lo, hi = 1024, 2048
        stt = st["stp"].tile([128, hi - lo], F32, tag="st",
                             padded_shape=[128, 1024])
        pieces = ([(lo, 512), (512, hi)] if (h, r, chunk) == (0, 0, 0)
                  else [(lo, hi)])
        for (alo, ahi) in pieces:
            for n0 in range(alo, ahi, 512):
                n1 = min(n0 + 512, ahi)
                nc.tensor.matmul(
                    stt[:, n0 - lo:n1 - lo],
                    kT[hp:hp + DH, hj, s0:s0 + 128],
                    qT[hp:hp + DH, hj, n0:n1],
                    start=True, stop=True,
                )
            nc.scalar.activation(
                ep[:, alo - s0:ahi - s0], stt[:, alo - lo:ahi - lo],
                EXP, scale=EXP_SCALE,
            )
        if chunk == 0:
            nc.gpsimd.tensor_mul(ep[:, 0:128], ep[:, 0:128], trim[:])

    def emit_s(h, r0=0, r1=NT):
        for r in range(r0, r1):
            emit_s_chunk(h, r, 0)
            if r < 8:
                emit_s_chunk(h, r, 1)

    def pv_quad(h, q0, nq):
        """PV psum groups for nq query tiles sharing one PSUM tile, with
        a single strided reciprocal + broadcast-multiply normalize.  The
        0.25 fold lives in the host-side W_v scale, so the normalize is
        a pure multiply and the per-tile DVE chain (two ops per query
        tile) collapses to two ops per quad."""
        v, yn = st["v"], st["yn"]
        par = (h // 2) % 2
        hq = h % 2
        ya = st["flex"].tile([128, nq, DH + 1], F32, tag="fx",
                             name=f"y{h}_{q0}", padded_shape=[128, 4, DH + 1])
        for qi in range(nq):
            qt = q0 + qi
            for r in range(qt + 1):
                ep = st["ep"][(h, r)]
                off = 128 * (qt - r)
                nc.tensor.matmul(
                    ya[:, qi, :], ep[:, off:off + 128], v[:, r, h, :],
                    start=(r == 0), stop=(r == qt),
                )
        rec = st["sp"].tile([128, nq], F32, tag="rec", name=f"rec{h}_{q0}",
                            padded_shape=[128, 4])
        nc.vector.reciprocal(rec[:], ya[:, :, DH])
        rb = rec[:].unsqueeze(2).broadcast_to((128, nq, DH))
        nc.vector.tensor_tensor(
            yn[:, par, q0:q0 + nq, hq, :], ya[:, :, 0:DH], rb, MUL,
        )

    def emit_pv(h, qt0=0, qt1=NT):
        for q0 in range(qt0, qt1, 4):
            pv_quad(h, q0, min(4, qt1 - q0))
        if qt1 == NT:
            for r in range(NT):
                del st["ep"][(h, r)]

    def tp_tile(j, qt):
        tp = st["flex"].tile([128, 128], F16, tag="fx", name=f"tp{j}_{qt}")
        nc.tensor.transpose(tp[:], st["yn"][:, j % 2, qt, :, :], st["ident"][:])
        nc.vector.tensor_copy(
            st["yT"][j][:, 128 * qt:128 * (qt + 1)], tp[:])

    def emit_tp(j, q0=0, q1=NT):
        if q0 == 0:
            st["yT"][j] = st["ytp"].tile([128, T], F16, tag="yT",
                                         name=f"yT{j}")
        for qt in range(q0, q1):
            tp_tile(j, qt)

    def emit_tail(qt0=0, qt1=NT):
        # Software-pipelined tail, lag 2: stage 1 at qt = PV(7, qt) +
        # pair-3 transpose into yT3[qt]; stage 2 at qt-2 = the WHOLE
        # out projection (one 8-matmul psum group per qt) -> psum->sbuf
        # copies on ACT/DVE -> DMA out.  Range form so the schedule can
        # weave tail stages between s(7) chunks: oj(qt) starts as soon
        # as exp(7, qt) lands instead of after the whole s(7) block.
        woT = st["woT"]

        def oj_qt(q):
            # Both 512-halves accumulate in one 2-bank stp tile (stp is
            # score-free by the tail); the PSUM->SBUF copies split across
            # ACT (idle after the last exp) and DVE so neither serializes
            # the per-qt chain.
            ps = st["stp"].tile([128, 1024], F32, tag="st", name=f"ob{q}")
            qs = slice(128 * q, 128 * (q + 1))
            for d in range(2):
                ds = slice(512 * d, 512 * (d + 1))
                for j in range(4):
                    nc.tensor.matmul(
                        ps[:, ds], st["yT"][j][:, qs], woT[:, j, ds],
                        start=(j == 0), stop=(j == 3),
                    )
                # d=0 copy on ACT (mostly idle by the tail) overlaps the
                # d=1 matmuls; d=1 copy on DVE so neither serializes.
                ob = st["lp"].tile([128, 512], F16, tag="ob",
                                   name=f"ob{q}_{d}", bufs=4)
                if d == 0:
                    nc.scalar.copy(ob[:], ps[:, 0:512])
                else:
                    nc.vector.tensor_copy(ob[:], ps[:, 512:1024])
                qu = nc.sync if d == 0 else nc.gpsimd
                orow = out_d[128 * q:128 * (q + 1), 512 * d:512 * (d + 1)]
                if q == NT - 1:
                    # Final group: halve the last transfers so the
                    # end-of-kernel DMA+sem chain is shorter.
                    qu.dma_start(orow[:, 0:256], ob[:, 0:256])
                    qu2 = nc.gpsimd if d == 0 else nc.sync
                    qu2.dma_start(orow[:, 256:512], ob[:, 256:512])
                else:
                    qu.dma_start(orow, ob[:])

        for qt in range(qt0, qt1):
            pv_quad(7, qt, 1)
            if qt0 == 0 and qt == 0:
                st["yT"][3] = st["ytp"].tile([128, T], F16, tag="yT",
                                             name="yT3")
            tp_tile(3, qt)
            if qt >= 2:
                oj_qt(qt - 2)
        if qt1 == NT:
            oj_qt(NT - 2)
            oj_qt(NT - 1)
            for r in range(NT):
                del st["ep"][(7, r)]

    with (
        tc.tile_pool(name="persist", bufs=1) as pp,
        tc.tile_pool(name="expp", bufs=3) as epool,
        tc.tile_pool(name="psum_st", bufs=3, space="PSUM") as stp,
        tc.tile_pool(name="psum_fx", bufs=2, space="PSUM") as flex,
        tc.tile_pool(name="small", bufs=4) as sp,
    ):
        st["qT"] = pp.tile([128, NJ, T], F16, tag="qT", name="qT")
        st["kT"] = pp.tile([128, NJ, T], F16, tag="kT", name="kT")
        st["v"] = pp.tile([128, NT, HPC, DH + 1], F16, tag="v", name="v")
        st["trim"] = pp.tile([128, 128], F16, tag="trim", name="trim")
        st["ident"] = pp.tile([128, 128], F16, tag="ident", name="ident")
        # normalized y staging: [pair parity, qt, head parity, dh]
        st["yn"] = pp.tile([128, 2, NT, 2, DH], F16, tag="yn", name="yn")
        st["epool"], st["stp"], st["flex"], st["sp"] = epool, stp, flex, sp
        st["ep"], st["yT"] = {}, {}

        warm = pp.tile([1, DH], F16, tag="warm", name="warm")
        nc.gpsimd.memset(warm[:], 1.0)
        nc.gpsimd.memset(st["v"][:, :, :, DH:DH + 1], DEN_COL)
        nc.scalar.activation(warm[:], warm[:], EXP, scale=1.0)

        # phase-A pools, closed by schedule markers (LIFO: wqk on top)
        wqk_ctx = tc.tile_pool(name="wqkpool", bufs=1)
        wqk = wqk_ctx.__enter__()
        xp_ctx = tc.tile_pool(name="x8pool", bufs=1)
        xp = xp_ctx.__enter__()
        wv_ctx = tc.tile_pool(name="wvpool", bufs=1)
        wv = wv_ctx.__enter__()

        st["x8h"] = xp.tile([128, 4, 2, T], F8, tag="x8h", name="x8h")
        st["x8l"] = xp.tile([128, 4, 2, T], F8, tag="x8l", name="x8l")
        st["w8"] = {nm: wqk.tile([128, 4, 2, DQ], F8, tag=nm, name=nm)
                    for nm in ("wqh", "wql", "wkh", "wkl")}
        st["w8v"] = {nm: wv.tile([128, 4, 2, DQ], F8, tag=nm, name=nm)
                     for nm in ("wvh", "wvl")}

        # DMA order: wq first, then x8 c-ascending with wk hoisted after
        # c1 (K_c0 runs ~4 groups in), wv/trim/ident late.  Full-tensor
        # transfers: each dma_start costs ~565ns of SP sequencer, so
        # finer slicing delays later chunks more than it helps the first.
        nc.sync.dma_start(st["w8"]["wqh"][:], w8_d["wqh"])
        nc.gpsimd.dma_start(st["w8"]["wql"][:], w8_d["wql"])
        for c in range(1):
            nc.sync.dma_start(
                st["x8h"][:, :, :, 512 * c:512 * (c + 1)], x8h_d[c])
            nc.gpsimd.dma_start(
                st["x8l"][:, :, :, 512 * c:512 * (c + 1)], x8l_d[c])
        nc.sync.dma_start(st["w8"]["wkh"][:], w8_d["wkh"])
        nc.gpsimd.dma_start(st["w8"]["wkl"][:], w8_d["wkl"])
        for c in range(1, 4):
            nc.sync.dma_start(
                st["x8h"][:, :, :, 512 * c:512 * (c + 1)], x8h_d[c])
            nc.gpsimd.dma_start(
                st["x8l"][:, :, :, 512 * c:512 * (c + 1)], x8l_d[c])
        nc.gpsimd.dma_start(st["trim"][:], tm_d)
        nc.sync.dma_start(st["w8v"]["wvh"][:], w8_d["wvh"])
        nc.gpsimd.dma_start(st["w8v"]["wvl"][:], w8_d["wvl"])
        nc.sync.dma_start(st["ident"][:], id_d)

        late_ctxs = []
        for op in SCHEDULE:
            kind = op[0]
            if kind == "qk":
                emit_qk(op[1])
            elif kind == "qkg":
                emit_qkg(op[1], op[2], op[3])
            elif kind == "s1":
                for r in range(op[2], op[3]):
                    emit_s_chunk(op[1], r, 0)
            elif kind == "s2":
                for r in range(op[2], op[3]):
                    emit_s_chunk(op[1], r, 1)
            elif kind == "s":
                emit_s(op[1], op[2], op[3])
            elif kind == "vp":
                emit_vp(op[1], *op[2:])
            elif kind == "pv":
                emit_pv(op[1], *op[2:])
            elif kind == "tp":
                emit_tp(op[1], *op[2:])
            elif kind == "tail":
                emit_tail(*op[1:])
            elif kind == "close_wqk":
                wqk_ctx.__exit__(None, None, None)
            elif kind == "close_x8":
                wv_ctx.__exit__(None, None, None)
                xp_ctx.__exit__(None, None, None)
            elif kind == "open_ytp":
                # Right-side pool: independent of the left-side stack
                # ordering (x8/wqk close at their own times).
                ytp_ctx = tc.tile_pool(name="ytp", bufs=4, side="right")
                st["ytp"] = ytp_ctx.__enter__()
                late_ctxs.append(ytp_ctx)
            elif kind == "open_late":
                lp_ctx = tc.tile_pool(name="late", bufs=1)
                lp = lp_ctx.__enter__()
                late_ctxs.append(lp_ctx)
                st["lp"] = lp
                st["woT"] = lp.tile([128, NJ, D], F16, tag="woT", name="woT")
                nc.sync.dma_start(
                    st["woT"][:], wo_d.rearrange("(j p) n -> p j n", p=128))
            else:
                raise ValueError(op)
        for ctx in reversed(late_ctxs):
            ctx.__exit__(None, None, None)


def build_nc():
    nc = bacc.Bacc("TRN2", target_bir_lowering=False, debug=False)
    with tile.TileContext(nc) as tc:
        _body(tc)
    nc.compile()
    return nc


_nc_cache = None


def _get_nc():
    global _nc_cache
    if _nc_cache is None:
        _nc_cache = build_nc()
    return _nc_cache


def make_in_maps(x, W_q, W_k, W_v, W_o):
    x = np.asarray(x, dtype=np.float32)
    W_q = np.asarray(W_q, dtype=np.float32)
    W_k = np.asarray(W_k, dtype=np.float32)
    W_v = np.asarray(W_v, dtype=np.float32)
    W_o = np.asarray(W_o, dtype=np.float32)
    in_maps = []
    for c in range(8):
        b, g = divmod(c, 2)
        sl = slice(DQ * g, DQ * (g + 1))
        im = {"trimask": TRIMASK, "ident": IDENT}
        xs = np.ascontiguousarray(x[b].T) * X_SCALE
        xh, xl = _split8(xs)
        for nm, a in (("x8h", xh), ("x8l", xl)):
            im[nm] = np.ascontiguousarray(
                a.reshape(4, 2, 128, 4, 512).transpose(3, 2, 0, 1, 4)
            )
        for nm, W, wsc in (("wq", W_q, W_SCALE), ("wk", W_k, W_SCALE),
                           ("wv", W_v, W_SCALE_V)):
            Ws = np.ascontiguousarray(W[sl].T) * wsc
            wh, wl = _split8(Ws)
            for suf, a in (("h", wh), ("l", wl)):
                im[nm + suf] = np.ascontiguousarray(
                    a.reshape(4, 2, 128, DQ).transpose(2, 0, 1, 3)
                )
        im["wot"] = np.ascontiguousarray(W_o[:, sl].T).astype(np.float16)
        in_maps.append(im)
    return in_maps


def kernel(x, W_q, W_k, W_v, W_o, b_o):
    global LAST
    nc = _get_nc()
    in_maps = make_in_maps(x, W_q, W_k, W_v, W_o)
    res = bass_utils.run_bass_kernel_spmd(
        nc, in_maps, core_ids=list(range(8)), trace=TRACE
    )
    LAST = res
    parts = [np.asarray(res.results[c]["out"], dtype=np.float32) for c in range(8)]
    b_o = np.asarray(b_o, dtype=np.float32)
    out = np.stack([parts[2 * b] + parts[2 * b + 1] for b in range(4)])
    out *= OUT_DESCALE
    out += b_o[None, None, :]
    return out.astype(np.float32)

